# revision 9
# baseline (speedup 1.0000x reference)
"""Trainium2 Bass kernel for nn_Attend_584115552611.

Attention B=4, H=16, N=2048, D=64 fp32 with the "swap" quirk:
when swap is truthy, attn probs of batches 0,1 are reused for batches 2,3
(each batch keeps its own v).  We therefore compute one softmax(QK^T) per
(qk-batch, head) "pair-unit" and apply it to two v tensors at once by
packing [v_b | v_{b+2}] into the 128 stationary PE columns.

Sharding: 32 pair-units (2 qk-batches x 16 heads) spread over 8 cores,
4 units per core (data/head parallel, no collectives).
"""

import sys
import functools

import numpy as np

for _p in ("/opt/trn_rl_repo",):
    if _p not in sys.path:
        sys.path.insert(0, _p)

import bass_rust
import concourse.bass as bass
import concourse.tile as tile
from concourse import mybir
from concourse.masks import make_identity

B, H, N, D = 4, 16, 2048, 64
N_CORES = 8
FP32 = mybir.dt.float32
BF16 = mybir.dt.bfloat16


def _split_excess_waits(nc, maxw=1):
    """This walrus build rejects instructions carrying more than one sync
    wait: spread excess waits onto inserted same-engine NOPs just before
    the offending instruction (engine queues are in-order, so semantics
    are unchanged)."""
    nid = 0
    for f in nc.m.functions:
        for bb in f.blocks:
            out = []
            changed = False
            for inst in bb.instructions:
                si = inst.sync_info
                waits = list(si.on_wait) if si and si.on_wait else []
                if len(waits) > maxw:
                    changed = True
                    for w in waits[:-maxw]:
                        nid += 1
                        nop = mybir.InstNoOp(name=f"I-waitsplit-{nid}")
                        nop.engine = inst.engine
                        nop.sync_info = bass_rust.SyncInfo(on_wait=[w], on_update=[])
                        out.append(nop)
                    si.on_wait = waits[-maxw:]
                out.append(inst)
            if changed:
                bb.instructions = out


def build_attn_program(n_units, n_ctx=N, d=D):
    """One softmax(q k^T * d^-0.5) per unit applied to TWO v tensors.

    DRAM params (per core):
      qk  [U, 2, n_ctx, d]  f32   (q, k)
      vv  [U, 2, n_ctx, d]  f32   (v for out slot 0 and slot 1)
      out [U, 2, n_ctx, d]  f32
    """
    assert d == 64 and n_ctx % 512 == 0
    T = n_ctx // 128          # k/q tiles of 128 rows
    NCH = n_ctx // 512        # 512-wide q chunks
    scale = float(d) ** -0.5

    nc = bass.Bass()
    qk = nc.declare_dram_parameter("qk", [n_units, 2, n_ctx, d], FP32, isOutput=False)
    vv = nc.declare_dram_parameter("vv", [n_units, 2, n_ctx, d], FP32, isOutput=False)
    out = nc.declare_dram_parameter("out", [n_units, 2, n_ctx, d], FP32, isOutput=True)

    with tile.TileContext(nc) as tc:
        with (
            tc.tile_pool(name="singles", bufs=1) as singles,
            tc.tile_pool(name="ins", bufs=2) as ins_pool,
            tc.tile_pool(name="tr", bufs=2) as tr_pool,
            tc.tile_pool(name="pt", bufs=2) as pt_pool,
            tc.tile_pool(name="sig", bufs=4) as sig_pool,
            tc.tile_pool(name="outs", bufs=2) as outs_pool,
            tc.tile_pool(name="qk_ps", bufs=2, space="PSUM") as qk_ps_pool,
            tc.tile_pool(name="av_ps", bufs=1, space="PSUM") as av_ps_pool,
            tc.tile_pool(name="sum_ps", bufs=1, space="PSUM") as sum_ps_pool,
            tc.tile_pool(name="tp_ps", bufs=2, space="PSUM") as tp_ps_pool,
        ):
            ident_bf = singles.tile([128, 128], BF16)
            make_identity(nc, ident_bf)
            ident_f32 = singles.tile([128, 128], FP32)
            make_identity(nc, ident_f32)
            ones_bf = singles.tile([128, 128], BF16)
            nc.vector.memset(ones_bf, 1.0)

            for u in range(n_units):
                # ---- loads (fp32 -> bf16 cast during SWDGE DMA) ----
                q_nat = ins_pool.tile([128, T, d], BF16, tag="q_nat")
                k_nat = ins_pool.tile([128, T, d], BF16, tag="k_nat")
                # vv_sb[p, t, w*64 + dd] = v_w[t*128 + p, dd]
                vv_sb = ins_pool.tile([128, T, 2 * d], BF16, tag="vv_sb")
                nc.gpsimd.dma_start(
                    out=q_nat, in_=qk[u, 0].rearrange("(t p) d -> p t d", p=128)
                )
                nc.gpsimd.dma_start(
                    out=k_nat, in_=qk[u, 1].rearrange("(t p) d -> p t d", p=128)
                )
                for w in range(2):
                    nc.gpsimd.dma_start(
                        out=vv_sb[:, :, w * d : (w + 1) * d],
                        in_=vv[u, w].rearrange("(t p) d -> p t d", p=128),
                    )

                # ---- transposes: qT (replicated halves) and kT (stacked) ----
                # qT_rep[dd, t, r] = q[t*128 + r, dd], same in dd+64
                qT_rep = tr_pool.tile([128, T, 128], BF16, tag="qT")
                # kT_st[dd + 64*(t odd), t//2, r] = k[t*128 + r, dd]
                kT_st = tr_pool.tile([128, T // 2, 128], BF16, tag="kT")
                for t in range(T):
                    tp = tp_ps_pool.tile([128, 128], BF16, tag="tp")
                    nc.tensor.transpose(tp[0:64, :], q_nat[:, t, :], ident_bf)
                    nc.vector.tensor_copy(out=qT_rep[0:64, t, :], in_=tp[0:64, :])
                    nc.vector.tensor_copy(out=qT_rep[64:128, t, :], in_=tp[0:64, :])
                for j in range(T // 2):
                    tp = tp_ps_pool.tile([128, 128], BF16, tag="tp")
                    nc.tensor.transpose(
                        tp, k_nat[:, 2 * j : 2 * j + 2, :], ident_bf
                    )
                    nc.vector.tensor_copy(out=kT_st[:, j, :], in_=tp)

                # ---- main loop over 512-wide q chunks ----
                out_nat = outs_pool.tile([128, T, 2, d], FP32, tag="out_nat")
                for c in range(NCH):
                    qs = c * 4  # first q-tile of this chunk
                    # P^T for this chunk: [k-in-tile, k-tile, q-in-chunk]
                    pT = pt_pool.tile([128, T, 512], BF16, tag="pT")
                    for j in range(T // 2):
                        ps = qk_ps_pool.tile([128, 1024], FP32, tag="qk")
                        # row-group 0: k-tile 2j ; row-group 1: k-tile 2j+1
                        nc.tensor.matmul(
                            ps[:, 0:512],
                            lhsT=kT_st[0:64, j, :],
                            rhs=qT_rep[0:64, qs : qs + 4, :],
                            start=True,
                            stop=True,
                        )
                        nc.tensor.matmul(
                            ps[:, 512:1024],
                            lhsT=kT_st[64:128, j, :],
                            rhs=qT_rep[64:128, qs : qs + 4, :],
                            start=True,
                            stop=True,
                        )
                        # exp(scale * scores) for both k-tiles at once
                        nc.scalar.activation(
                            out=pT[:, 2 * j : 2 * j + 2, :],
                            in_=ps,
                            func=mybir.ActivationFunctionType.Exp,
                            scale=scale,
                        )

                    # ---- AV (v-pair stationary) + column sums (ones) ----
                    av = av_ps_pool.tile([128, 512], FP32, tag="av")
                    # ones[128,128] stationary -> sigma replicated on all partitions
                    sm = sum_ps_pool.tile([128, 512], FP32, tag="sm")
                    for t in range(T):
                        nc.tensor.matmul(
                            av,
                            lhsT=vv_sb[:, t, :],
                            rhs=pT[:, t, :],
                            start=(t == 0),
                            stop=(t == T - 1),
                        )
                        nc.tensor.matmul(
                            sm,
                            lhsT=ones_bf,
                            rhs=pT[:, t, :],
                            start=(t == 0),
                            stop=(t == T - 1),
                        )

                    # ---- sigma -> reciprocal -> normalize ----
                    rec_bc = sig_pool.tile([128, 512], FP32, tag="rec_bc")
                    nc.vector.reciprocal(out=rec_bc, in_=sm)
                    oT = sig_pool.tile([128, 512], FP32, tag="oT")
                    nc.vector.tensor_mul(oT, av, rec_bc)

                    # ---- transpose out^T back to natural [q, (w,d)] ----
                    for Tq in range(4):
                        tp2 = tp_ps_pool.tile([128, 128], FP32, tag="tp")
                        nc.tensor.transpose(
                            tp2, oT[:, 128 * Tq : 128 * Tq + 128], ident_f32
                        )
                        nc.vector.tensor_copy(
                            out=out_nat[:, qs + Tq, :, :], in_=tp2
                        )

                for w in range(2):
                    nc.sync.dma_start(
                        out=out[u, w].rearrange("(t p) d -> p t d", p=128),
                        in_=out_nat[:, :, w, :],
                    )

    _split_excess_waits(nc)
    return nc


@functools.lru_cache(maxsize=4)
def _get_program(n_units, n_ctx):
    return build_attn_program(n_units, n_ctx)


def _get_runner(n_units, n_ctx):
    """Build the bass program once and return a cached jitted SPMD runner:
    runner(concat_qk, concat_vv) -> concat_out  (axis 0 = cores*units)."""
    import jax
    from jax.experimental.shard_map import shard_map
    from jax.sharding import Mesh, PartitionSpec
    from concourse import bass2jax

    bass2jax.install_neuronx_cc_hook()
    nc = _get_program(n_units, n_ctx)

    in_names, out_names, out_avals, zero_shapes = [], [], [], []
    for alloc in nc.m.functions[0].allocations:
        if not isinstance(alloc, mybir.MemoryLocationSet):
            continue
        name = alloc.memorylocations[0].name
        if alloc.kind == "ExternalInput":
            if nc.partition_id_tensor is None or name != nc.partition_id_tensor.name:
                in_names.append(name)
        elif alloc.kind == "ExternalOutput":
            out_names.append(name)
            shape = tuple(alloc.tensor_shape)
            dtype = mybir.dt.np(alloc.dtype)
            out_avals.append(jax.core.ShapedArray(shape, dtype))
            zero_shapes.append((shape, dtype))
    assert in_names == ["qk", "vv"] and out_names == ["out"]
    n_params = len(in_names)
    all_names = in_names + out_names
    if nc.partition_id_tensor is not None:
        all_names.append(nc.partition_id_tensor.name)

    def _body(*args):
        operands = list(args)
        if nc.partition_id_tensor is not None:
            operands.append(bass2jax.partition_id_tensor())
        outs = bass2jax._bass_exec_p.bind(
            *operands,
            out_avals=tuple(out_avals),
            in_names=tuple(all_names),
            out_names=tuple(out_names),
            lowering_input_output_aliases=(),
            sim_require_finite=True,
            sim_require_nnan=True,
            nc=nc,
        )
        return tuple(outs)

    devices = jax.devices()[:N_CORES]
    mesh = Mesh(np.asarray(devices), ("core",))
    n_outs = len(out_names)
    sharded = jax.jit(
        shard_map(
            _body,
            mesh=mesh,
            in_specs=(PartitionSpec("core"),) * (n_params + n_outs),
            out_specs=(PartitionSpec("core"),) * n_outs,
            check_rep=False,
        ),
        keep_unused=True,
    )

    def runner(qk_all, vv_all):
        zeros = [
            np.zeros((N_CORES * s[0], *s[1:]), dt) for (s, dt) in zero_shapes
        ]
        (out_all,) = sharded(qk_all, vv_all, *zeros)
        return np.asarray(out_all)

    runner.sharded = sharded
    runner.mesh = mesh
    runner.zero_shapes = zero_shapes
    return runner


_RUNNERS = {}


def _run_units(unit_specs, q, k, v, n_ctx):
    """unit_specs: list of (qk_batch, head, v_batch0, v_batch1)."""
    n_units = len(unit_specs) // N_CORES
    assert n_units * N_CORES == len(unit_specs)
    key = (n_units, n_ctx)
    if key not in _RUNNERS:
        _RUNNERS[key] = _get_runner(n_units, n_ctx)
    runner = _RUNNERS[key]

    qk_all = np.empty((N_CORES * n_units, 2, n_ctx, D), np.float32)
    vv_all = np.empty((N_CORES * n_units, 2, n_ctx, D), np.float32)
    for i, (bq, h, b0, b1) in enumerate(unit_specs):
        qk_all[i, 0] = q[bq, h]
        qk_all[i, 1] = k[bq, h]
        vv_all[i, 0] = v[b0, h]
        vv_all[i, 1] = v[b1, h]

    out_all = runner(qk_all, vv_all).reshape(N_CORES * n_units, 2, n_ctx, D)

    out = np.empty((B, H, n_ctx, D), np.float32)
    for i, (bq, h, b0, b1) in enumerate(unit_specs):
        out[b0, h] = out_all[i, 0]
        if b1 != b0:
            out[b1, h] = out_all[i, 1]
    return out


def kernel(q, k, v, swap):
    q = np.ascontiguousarray(np.asarray(q, dtype=np.float32))
    k = np.ascontiguousarray(np.asarray(k, dtype=np.float32))
    v = np.ascontiguousarray(np.asarray(v, dtype=np.float32))
    swap_val = int(np.asarray(swap).reshape(-1)[0])

    n_ctx = q.shape[2]
    if swap_val:
        # 32 pair-units: attn of (b, h) applied to v[b] and v[b + B//2]
        specs = [(bq, h, bq, bq + B // 2) for bq in range(B // 2) for h in range(H)]
    else:
        # 64 independent units (2nd v slot duplicates the 1st)
        specs = [(b, h, b, b) for b in range(B) for h in range(H)]
    return _run_units(specs, q, k, v, n_ctx)


if __name__ == "__main__":
    rng = np.random.default_rng(0)
    q = rng.standard_normal((B, H, N, D), dtype=np.float32)
    k = rng.standard_normal((B, H, N, D), dtype=np.float32)
    v = rng.standard_normal((B, H, N, D), dtype=np.float32)
    o = kernel(q, k, v, 1)
    print("out", o.shape, o.dtype, float(np.abs(o).mean()))


# revision 18
# speedup vs baseline: 244.0598x; 244.0598x over previous
"""Trainium2 Bass kernel for nn_Attend_584115552611.

Attention B=4, H=16, N=2048, D=64 fp32 with the "swap" quirk:
when swap is truthy, attn probs of batches 0,1 are reused for batches 2,3
(each batch keeps its own v).  We therefore compute one softmax(QK^T) per
(qk-batch, head) "pair-unit" and apply it to two v tensors at once by
packing [v_b | v_{b+2}] into the 128 stationary PE columns.

Sharding: 32 pair-units (2 qk-batches x 16 heads) spread over 8 cores,
4 units per core (data/head parallel, no collectives).
"""

import sys
import functools

import numpy as np

for _p in ("/opt/trn_rl_repo",):
    if _p not in sys.path:
        sys.path.insert(0, _p)

import bass_rust
import concourse.bass as bass
import concourse.tile as tile
from concourse import mybir
from concourse.masks import make_identity

B, H, N, D = 4, 16, 2048, 64
N_CORES = 8
FP32 = mybir.dt.float32
BF16 = mybir.dt.bfloat16
FP32R = mybir.dt.float32r


def _split_excess_waits(nc, maxw=1):
    """This walrus build rejects instructions carrying more than one sync
    wait: spread excess waits onto inserted same-engine NOPs just before
    the offending instruction (engine queues are in-order, so semantics
    are unchanged)."""
    nid = 0
    for f in nc.m.functions:
        for bb in f.blocks:
            out = []
            changed = False
            for inst in bb.instructions:
                si = inst.sync_info
                waits = list(si.on_wait) if si and si.on_wait else []
                if len(waits) > maxw:
                    changed = True
                    for w in waits[:-maxw]:
                        nid += 1
                        nop = mybir.InstNoOp(name=f"I-waitsplit-{nid}")
                        nop.engine = inst.engine
                        nop.sync_info = bass_rust.SyncInfo(on_wait=[w], on_update=[])
                        out.append(nop)
                    si.on_wait = waits[-maxw:]
                out.append(inst)
            if changed:
                bb.instructions = out


def build_attn_program(n_units, n_ctx=N, d=D):
    """One softmax(q k^T * d^-0.5) per unit applied to TWO v tensors.

    DRAM params (per core):
      qk  [U, 2, n_ctx, d]  f32   (q, k)
      vv  [U, 2, n_ctx, d]  f32   (v for out slot 0 and slot 1)
      out [U, 2, n_ctx, d]  f32
    """
    assert d == 64 and n_ctx % 512 == 0
    T = n_ctx // 128          # k/q tiles of 128 rows
    NCH = n_ctx // 512        # 512-wide q chunks
    scale = float(d) ** -0.5

    nc = bass.Bass()
    qk = nc.declare_dram_parameter("qk", [n_units, 2, n_ctx, d], FP32, isOutput=False)
    vv = nc.declare_dram_parameter("vv", [n_units, 2, n_ctx, d], FP32, isOutput=False)
    out = nc.declare_dram_parameter("out", [n_units, 2, n_ctx, d], FP32, isOutput=True)

    with tile.TileContext(nc) as tc:
        with (
            tc.tile_pool(name="singles", bufs=1) as singles,
            tc.tile_pool(name="ins", bufs=2) as ins_pool,
            tc.tile_pool(name="tr", bufs=2) as tr_pool,
            tc.tile_pool(name="pt", bufs=2) as pt_pool,
            tc.tile_pool(name="sig", bufs=4) as sig_pool,
            tc.tile_pool(name="outs", bufs=2) as outs_pool,
            tc.tile_pool(name="qk_ps", bufs=2, space="PSUM") as qk_ps_pool,
            tc.tile_pool(name="av_ps", bufs=1, space="PSUM") as av_ps_pool,
            tc.tile_pool(name="sum_ps", bufs=1, space="PSUM") as sum_ps_pool,
            tc.tile_pool(name="tp_ps", bufs=2, space="PSUM") as tp_ps_pool,
        ):
            ident_f32 = singles.tile([128, 128], FP32)
            make_identity(nc, ident_f32)
            ident_r = singles.tile([128, 128], FP32R)
            nc.vector.tensor_copy(out=ident_r, in_=ident_f32)
            ones_bf = singles.tile([128, 128], BF16)
            nc.vector.memset(ones_bf, 1.0)

            for u in range(n_units):
                # ---- loads (SWDGE DMA casts fp32 -> fp32r / bf16) ----
                # qk_nat[:, 0:T] = q tiles, [:, T:2T] = k tiles
                qk_nat = ins_pool.tile([128, 2 * T, d], FP32R, tag="qk_nat")
                q_nat = qk_nat[:, 0:T, :]
                k_nat = qk_nat[:, T : 2 * T, :]
                # vv_sb[p, t, w*64 + dd] = v_w[t*128 + p, dd]
                vv_sb = ins_pool.tile([128, T, 2 * d], BF16, tag="vv_sb")
                nc.gpsimd.dma_start(
                    out=qk_nat, in_=qk[u].rearrange("w (t p) d -> p (w t) d", p=128)
                )
                for w in range(2):
                    nc.gpsimd.dma_start(
                        out=vv_sb[:, :, w * d : (w + 1) * d],
                        in_=vv[u, w].rearrange("(t p) d -> p t d", p=128),
                    )

                # ---- transposes: qT (replicated halves) and kT (stacked) ----
                # qT_rep[dd, t, r] = q[t*128 + r, dd], same in dd+64
                qT_rep = tr_pool.tile([128, T, 128], FP32R, tag="qT")
                # kT_st[dd + 64*(t odd), t//2, r] = k[t*128 + r, dd]
                kT_st = tr_pool.tile([128, T // 2, 128], FP32R, tag="kT")
                for t in range(T):
                    tp = tp_ps_pool.tile([128, 128], FP32R, tag="tp")
                    nc.tensor.transpose(tp[0:64, :], q_nat[:, t, :], ident_r)
                    nc.vector.tensor_copy(out=qT_rep[0:64, t, :], in_=tp[0:64, :])
                nc.vector.tensor_copy(
                    out=qT_rep[64:128, :, :], in_=qT_rep[0:64, :, :]
                )
                for j in range(T // 2):
                    tp = tp_ps_pool.tile([128, 128], FP32R, tag="tp")
                    nc.tensor.transpose(
                        tp, k_nat[:, 2 * j : 2 * j + 2, :], ident_r
                    )
                    nc.vector.tensor_copy(out=kT_st[:, j, :], in_=tp)

                # ---- main loop over 512-wide q chunks ----
                out_nat = outs_pool.tile([128, T, 2, d], FP32, tag="out_nat")
                for c in range(NCH):
                    qs = c * 4  # first q-tile of this chunk
                    # P^T for this chunk: [k-in-tile, k-tile, q-in-chunk]
                    pT = pt_pool.tile([128, T, 512], BF16, tag="pT")
                    for j in range(T // 2):
                        ps = qk_ps_pool.tile([128, 1024], FP32, tag="qk")
                        # row-group 0: k-tile 2j ; row-group 1: k-tile 2j+1
                        nc.tensor.matmul(
                            ps[:, 0:512],
                            lhsT=kT_st[0:64, j, :],
                            rhs=qT_rep[0:64, qs : qs + 4, :],
                            start=True,
                            stop=True,
                        )
                        nc.tensor.matmul(
                            ps[:, 512:1024],
                            lhsT=kT_st[64:128, j, :],
                            rhs=qT_rep[64:128, qs : qs + 4, :],
                            start=True,
                            stop=True,
                        )
                        # exp(scale * scores) for both k-tiles at once
                        nc.scalar.activation(
                            out=pT[:, 2 * j : 2 * j + 2, :],
                            in_=ps,
                            func=mybir.ActivationFunctionType.Exp,
                            scale=scale,
                        )

                    # ---- AV (v-pair stationary) + column sums (ones) ----
                    av = av_ps_pool.tile([128, 512], FP32, tag="av")
                    # ones[128,128] stationary -> sigma replicated on all partitions
                    sm = sum_ps_pool.tile([128, 512], FP32, tag="sm")
                    for t in range(T):
                        nc.tensor.matmul(
                            av,
                            lhsT=vv_sb[:, t, :],
                            rhs=pT[:, t, :],
                            start=(t == 0),
                            stop=(t == T - 1),
                        )
                        nc.tensor.matmul(
                            sm,
                            lhsT=ones_bf,
                            rhs=pT[:, t, :],
                            start=(t == 0),
                            stop=(t == T - 1),
                        )

                    # ---- sigma -> reciprocal -> normalize ----
                    rec_bc = sig_pool.tile([128, 512], FP32, tag="rec_bc")
                    nc.vector.reciprocal(out=rec_bc, in_=sm)
                    oT = sig_pool.tile([128, 512], FP32, tag="oT")
                    nc.vector.tensor_mul(oT, av, rec_bc)

                    # ---- transpose out^T back to natural [q, (w,d)] ----
                    for Tq in range(4):
                        tp2 = tp_ps_pool.tile([128, 128], FP32, tag="tp")
                        nc.tensor.transpose(
                            tp2, oT[:, 128 * Tq : 128 * Tq + 128], ident_f32
                        )
                        nc.vector.tensor_copy(
                            out=out_nat[:, qs + Tq, :, :], in_=tp2
                        )

                for w in range(2):
                    nc.sync.dma_start(
                        out=out[u, w].rearrange("(t p) d -> p t d", p=128),
                        in_=out_nat[:, :, w, :],
                    )

    _split_excess_waits(nc)
    return nc


@functools.lru_cache(maxsize=4)
def _get_program(n_units, n_ctx):
    return build_attn_program(n_units, n_ctx)


def _get_runner(n_units, n_ctx):
    """Build the bass program once and return a cached jitted SPMD runner:
    runner(concat_qk, concat_vv) -> concat_out  (axis 0 = cores*units)."""
    import jax
    from jax.experimental.shard_map import shard_map
    from jax.sharding import Mesh, PartitionSpec
    from concourse import bass2jax

    try:
        jax.config.update("jax_compilation_cache_dir", "/tmp/jax_neff_cache")
        jax.config.update("jax_persistent_cache_min_compile_time_secs", 10)
    except Exception:
        pass
    bass2jax.install_neuronx_cc_hook()
    nc = _get_program(n_units, n_ctx)

    in_names, out_names, out_avals, zero_shapes = [], [], [], []
    for alloc in nc.m.functions[0].allocations:
        if not isinstance(alloc, mybir.MemoryLocationSet):
            continue
        name = alloc.memorylocations[0].name
        if alloc.kind == "ExternalInput":
            if nc.partition_id_tensor is None or name != nc.partition_id_tensor.name:
                in_names.append(name)
        elif alloc.kind == "ExternalOutput":
            out_names.append(name)
            shape = tuple(alloc.tensor_shape)
            dtype = mybir.dt.np(alloc.dtype)
            out_avals.append(jax.core.ShapedArray(shape, dtype))
            zero_shapes.append((shape, dtype))
    assert in_names == ["qk", "vv"] and out_names == ["out"]
    n_params = len(in_names)
    all_names = in_names + out_names
    if nc.partition_id_tensor is not None:
        all_names.append(nc.partition_id_tensor.name)

    def _body(*args):
        operands = list(args)
        if nc.partition_id_tensor is not None:
            operands.append(bass2jax.partition_id_tensor())
        outs = bass2jax._bass_exec_p.bind(
            *operands,
            out_avals=tuple(out_avals),
            in_names=tuple(all_names),
            out_names=tuple(out_names),
            lowering_input_output_aliases=(),
            sim_require_finite=True,
            sim_require_nnan=True,
            nc=nc,
        )
        return tuple(outs)

    devices = jax.devices()[:N_CORES]
    mesh = Mesh(np.asarray(devices), ("core",))
    n_outs = len(out_names)
    sharded = jax.jit(
        shard_map(
            _body,
            mesh=mesh,
            in_specs=(PartitionSpec("core"),) * (n_params + n_outs),
            out_specs=(PartitionSpec("core"),) * n_outs,
            check_rep=False,
        ),
        keep_unused=True,
    )

    def runner(qk_all, vv_all):
        zeros = [
            np.zeros((N_CORES * s[0], *s[1:]), dt) for (s, dt) in zero_shapes
        ]
        (out_all,) = sharded(qk_all, vv_all, *zeros)
        return np.asarray(out_all)

    runner.sharded = sharded
    runner.mesh = mesh
    runner.zero_shapes = zero_shapes
    return runner


_RUNNERS = {}


def _run_units(unit_specs, q, k, v, n_ctx):
    """unit_specs: list of (qk_batch, head, v_batch0, v_batch1)."""
    n_units = len(unit_specs) // N_CORES
    assert n_units * N_CORES == len(unit_specs)
    key = (n_units, n_ctx)
    if key not in _RUNNERS:
        _RUNNERS[key] = _get_runner(n_units, n_ctx)
    runner = _RUNNERS[key]

    qk_all = np.empty((N_CORES * n_units, 2, n_ctx, D), np.float32)
    vv_all = np.empty((N_CORES * n_units, 2, n_ctx, D), np.float32)
    for i, (bq, h, b0, b1) in enumerate(unit_specs):
        qk_all[i, 0] = q[bq, h]
        qk_all[i, 1] = k[bq, h]
        vv_all[i, 0] = v[b0, h]
        vv_all[i, 1] = v[b1, h]

    out_all = runner(qk_all, vv_all).reshape(N_CORES * n_units, 2, n_ctx, D)

    out = np.empty((B, H, n_ctx, D), np.float32)
    for i, (bq, h, b0, b1) in enumerate(unit_specs):
        out[b0, h] = out_all[i, 0]
        if b1 != b0:
            out[b1, h] = out_all[i, 1]
    return out


def kernel(q, k, v, swap):
    q = np.ascontiguousarray(np.asarray(q, dtype=np.float32))
    k = np.ascontiguousarray(np.asarray(k, dtype=np.float32))
    v = np.ascontiguousarray(np.asarray(v, dtype=np.float32))
    swap_val = int(np.asarray(swap).reshape(-1)[0])

    n_ctx = q.shape[2]
    if swap_val:
        # 32 pair-units: attn of (b, h) applied to v[b] and v[b + B//2]
        specs = [(bq, h, bq, bq + B // 2) for bq in range(B // 2) for h in range(H)]
    else:
        # 64 independent units (2nd v slot duplicates the 1st)
        specs = [(b, h, b, b) for b in range(B) for h in range(H)]
    return _run_units(specs, q, k, v, n_ctx)


if __name__ == "__main__":
    rng = np.random.default_rng(0)
    q = rng.standard_normal((B, H, N, D), dtype=np.float32)
    k = rng.standard_normal((B, H, N, D), dtype=np.float32)
    v = rng.standard_normal((B, H, N, D), dtype=np.float32)
    o = kernel(q, k, v, 1)
    print("out", o.shape, o.dtype, float(np.abs(o).mean()))


# revision 21
# speedup vs baseline: 320.0484x; 1.3114x over previous
"""Trainium2 Bass kernel for nn_Attend_584115552611.

Attention B=4, H=16, N=2048, D=64 fp32 with the "swap" quirk:
when swap is truthy, attn probs of batches 0,1 are reused for batches 2,3
(each batch keeps its own v).  We therefore compute one softmax(QK^T) per
(qk-batch, head) "pair-unit" and apply it to two v tensors at once by
packing [v_b | v_{b+2}] into the 128 stationary PE columns.

Sharding: 32 pair-units (2 qk-batches x 16 heads) spread over 8 cores,
4 units per core (data/head parallel, no collectives).
"""

import sys
import functools

import numpy as np

for _p in ("/opt/trn_rl_repo",):
    if _p not in sys.path:
        sys.path.insert(0, _p)

import bass_rust
import concourse.bass as bass
import concourse.tile as tile
from concourse import mybir
from concourse.masks import make_identity

B, H, N, D = 4, 16, 2048, 64
N_CORES = 8
FP32 = mybir.dt.float32
BF16 = mybir.dt.bfloat16
FP32R = mybir.dt.float32r


def _split_excess_waits(nc, maxw=1):
    """This walrus build rejects instructions carrying more than one sync
    wait: spread excess waits onto inserted same-engine NOPs just before
    the offending instruction (engine queues are in-order, so semantics
    are unchanged)."""
    nid = 0
    for f in nc.m.functions:
        for bb in f.blocks:
            out = []
            changed = False
            for inst in bb.instructions:
                si = inst.sync_info
                waits = list(si.on_wait) if si and si.on_wait else []
                if len(waits) > maxw:
                    changed = True
                    for w in waits[:-maxw]:
                        nid += 1
                        nop = mybir.InstNoOp(name=f"I-waitsplit-{nid}")
                        nop.engine = inst.engine
                        nop.sync_info = bass_rust.SyncInfo(on_wait=[w], on_update=[])
                        out.append(nop)
                    si.on_wait = waits[-maxw:]
                out.append(inst)
            if changed:
                bb.instructions = out


def build_attn_program(n_units, n_ctx=N, d=D):
    """One softmax(q k^T * d^-0.5) per unit applied to TWO v tensors.

    The host pre-packs all operands into PE-friendly layouts, so the kernel
    does no transposes at all:
      qt  [U, 128, T*128]    f32  rows 0:64 = q^T (d-major), rows 64:128 = copy
                                  (replica feeds PE row-group 1)
      kt  [U, 128, T/2*128]  f32  rows 0:64 = k^T of even 128-row tiles,
                                  rows 64:128 = odd tiles (row-group packing)
      vv  [U, 128, T*128]    f32  [p, t*128 + w*64+dd] = v_w[t*128+p, dd]
      out [U, 128, n_ctx]    f32  out^T: [w*64+dd, q] (host transposes back)
    """
    assert d == 64 and n_ctx % 512 == 0
    T = n_ctx // 128          # k/q tiles of 128 rows
    NCH = n_ctx // 512        # 512-wide q chunks
    scale = float(d) ** -0.5

    nc = bass.Bass()
    qt = nc.declare_dram_parameter("qt", [n_units, 128, T * 128], FP32, isOutput=False)
    kt = nc.declare_dram_parameter(
        "kt", [n_units, 128, (T // 2) * 128], FP32, isOutput=False
    )
    vv = nc.declare_dram_parameter("vv", [n_units, 128, T * 128], FP32, isOutput=False)
    out = nc.declare_dram_parameter("out", [n_units, 128, n_ctx], FP32, isOutput=True)

    with tile.TileContext(nc) as tc:
        with (
            tc.tile_pool(name="singles", bufs=1) as singles,
            tc.tile_pool(name="ins", bufs=2) as ins_pool,
            tc.tile_pool(name="pt", bufs=2) as pt_pool,
            tc.tile_pool(name="sig", bufs=4) as sig_pool,
            tc.tile_pool(name="outs", bufs=2) as outs_pool,
            tc.tile_pool(name="qk_ps", bufs=3, space="PSUM") as qk_ps_pool,
            tc.tile_pool(name="av_ps", bufs=1, space="PSUM") as av_ps_pool,
            tc.tile_pool(name="sum_ps", bufs=1, space="PSUM") as sum_ps_pool,
        ):
            ones_bf = singles.tile([128, 128], BF16)
            nc.vector.memset(ones_bf, 1.0)

            for u in range(n_units):
                # ---- loads (contiguous; SWDGE DMA casts fp32 -> fp32r/bf16) ----
                qT_rep = ins_pool.tile([128, T, 128], FP32R, tag="qT")
                kT_st = ins_pool.tile([128, T // 2, 128], FP32R, tag="kT")
                vv_sb = ins_pool.tile([128, T, 128], BF16, tag="vv_sb")
                nc.gpsimd.dma_start(
                    out=qT_rep, in_=qt[u].rearrange("p (t r) -> p t r", t=T)
                )
                nc.gpsimd.dma_start(
                    out=kT_st, in_=kt[u].rearrange("p (j r) -> p j r", j=T // 2)
                )
                nc.gpsimd.dma_start(
                    out=vv_sb, in_=vv[u].rearrange("p (t r) -> p t r", t=T)
                )

                # ---- main loop: software-pipelined over 512-wide q chunks.
                # Emit QK+exp for chunk c BEFORE AV/sums for chunk c-1 so
                # ScalarE exps chunk c while the PE streams chunk c-1's
                # AV+sums (otherwise the two phases serialize).
                oT_unit = outs_pool.tile([128, NCH, 512], FP32, tag="oT_unit")
                pT_tiles = {}
                for c in range(NCH + 1):
                    if c < NCH:
                        qs = c * 4  # first q-tile of this chunk
                        # P^T for this chunk: [k-in-tile, k-tile, q-in-chunk]
                        pT = pt_pool.tile([128, T, 512], BF16, tag="pT")
                        pT_tiles[c] = pT
                        for j in range(T // 2):
                            ps = qk_ps_pool.tile([128, 1024], FP32, tag="qk")
                            # row-group 0: k-tile 2j ; row-group 1: k-tile 2j+1
                            nc.tensor.matmul(
                                ps[:, 0:512],
                                lhsT=kT_st[0:64, j, :],
                                rhs=qT_rep[0:64, qs : qs + 4, :],
                                start=True,
                                stop=True,
                            )
                            nc.tensor.matmul(
                                ps[:, 512:1024],
                                lhsT=kT_st[64:128, j, :],
                                rhs=qT_rep[64:128, qs : qs + 4, :],
                                start=True,
                                stop=True,
                            )
                            # exp(scale * scores) for both k-tiles at once
                            nc.scalar.activation(
                                out=pT[:, 2 * j : 2 * j + 2, :],
                                in_=ps,
                                func=mybir.ActivationFunctionType.Exp,
                                scale=scale,
                            )

                    if c == 0:
                        continue
                    cc = c - 1
                    pT = pT_tiles.pop(cc)

                    # ---- AV (v-pair stationary) + column sums (ones) ----
                    av = av_ps_pool.tile([128, 512], FP32, tag="av")
                    # ones[128,128] stationary -> sigma replicated on all partitions
                    sm = sum_ps_pool.tile([128, 512], FP32, tag="sm")
                    for t in range(T):
                        nc.tensor.matmul(
                            av,
                            lhsT=vv_sb[:, t, :],
                            rhs=pT[:, t, :],
                            start=(t == 0),
                            stop=(t == T - 1),
                        )
                        nc.tensor.matmul(
                            sm,
                            lhsT=ones_bf,
                            rhs=pT[:, t, :],
                            start=(t == 0),
                            stop=(t == T - 1),
                        )

                    # ---- sigma -> reciprocal -> normalize (out^T stays) ----
                    rec_bc = sig_pool.tile([128, 512], FP32, tag="rec_bc")
                    nc.vector.reciprocal(out=rec_bc, in_=sm)
                    nc.vector.tensor_mul(oT_unit[:, cc, :], av, rec_bc)

                nc.sync.dma_start(
                    out=out[u].rearrange("p (c q) -> p c q", c=NCH), in_=oT_unit
                )

    _split_excess_waits(nc)
    return nc


@functools.lru_cache(maxsize=4)
def _get_program(n_units, n_ctx):
    return build_attn_program(n_units, n_ctx)


def _get_runner(n_units, n_ctx):
    """Build the bass program once and return a cached jitted SPMD runner:
    runner(concat_qk, concat_vv) -> concat_out  (axis 0 = cores*units)."""
    import jax
    from jax.experimental.shard_map import shard_map
    from jax.sharding import Mesh, PartitionSpec
    from concourse import bass2jax

    try:
        jax.config.update("jax_compilation_cache_dir", "/tmp/jax_neff_cache")
        jax.config.update("jax_persistent_cache_min_compile_time_secs", 10)
    except Exception:
        pass
    bass2jax.install_neuronx_cc_hook()
    nc = _get_program(n_units, n_ctx)

    in_names, out_names, out_avals, zero_shapes = [], [], [], []
    for alloc in nc.m.functions[0].allocations:
        if not isinstance(alloc, mybir.MemoryLocationSet):
            continue
        name = alloc.memorylocations[0].name
        if alloc.kind == "ExternalInput":
            if nc.partition_id_tensor is None or name != nc.partition_id_tensor.name:
                in_names.append(name)
        elif alloc.kind == "ExternalOutput":
            out_names.append(name)
            shape = tuple(alloc.tensor_shape)
            dtype = mybir.dt.np(alloc.dtype)
            out_avals.append(jax.core.ShapedArray(shape, dtype))
            zero_shapes.append((shape, dtype))
    assert in_names == ["qt", "kt", "vv"] and out_names == ["out"]
    n_params = len(in_names)
    all_names = in_names + out_names
    if nc.partition_id_tensor is not None:
        all_names.append(nc.partition_id_tensor.name)

    def _body(*args):
        operands = list(args)
        if nc.partition_id_tensor is not None:
            operands.append(bass2jax.partition_id_tensor())
        outs = bass2jax._bass_exec_p.bind(
            *operands,
            out_avals=tuple(out_avals),
            in_names=tuple(all_names),
            out_names=tuple(out_names),
            lowering_input_output_aliases=(),
            sim_require_finite=True,
            sim_require_nnan=True,
            nc=nc,
        )
        return tuple(outs)

    devices = jax.devices()[:N_CORES]
    mesh = Mesh(np.asarray(devices), ("core",))
    n_outs = len(out_names)
    sharded = jax.jit(
        shard_map(
            _body,
            mesh=mesh,
            in_specs=(PartitionSpec("core"),) * (n_params + n_outs),
            out_specs=(PartitionSpec("core"),) * n_outs,
            check_rep=False,
        ),
        keep_unused=True,
    )

    def runner(qt_all, kt_all, vv_all):
        zeros = [
            np.zeros((N_CORES * s[0], *s[1:]), dt) for (s, dt) in zero_shapes
        ]
        (out_all,) = sharded(qt_all, kt_all, vv_all, *zeros)
        return np.asarray(out_all)

    runner.sharded = sharded
    runner.mesh = mesh
    runner.zero_shapes = zero_shapes
    return runner


_RUNNERS = {}


def _run_units(unit_specs, q, k, v, n_ctx):
    """unit_specs: list of (qk_batch, head, v_batch0, v_batch1)."""
    n_units = len(unit_specs) // N_CORES
    assert n_units * N_CORES == len(unit_specs)
    key = (n_units, n_ctx)
    if key not in _RUNNERS:
        _RUNNERS[key] = _get_runner(n_units, n_ctx)
    runner = _RUNNERS[key]

    T = n_ctx // 128
    NU = N_CORES * n_units
    qt_all = np.empty((NU, 128, T * 128), np.float32)
    kt_all = np.empty((NU, 128, (T // 2) * 128), np.float32)
    vv_all = np.empty((NU, 128, T * 128), np.float32)
    for i, (bq, h, b0, b1) in enumerate(unit_specs):
        qT = q[bq, h].T                      # [64, n_ctx], d-major
        qt_all[i, 0:64] = qT
        qt_all[i, 64:128] = qT               # replica feeds PE row-group 1
        kT = k[bq, h].T.reshape(64, T, 128)  # [dd, t, r]
        kt_all[i, 0:64] = kT[:, 0::2, :].reshape(64, -1)
        kt_all[i, 64:128] = kT[:, 1::2, :].reshape(64, -1)
        v0 = v[b0, h].reshape(T, 128, D)     # [t, p, dd]
        v1 = v[b1, h].reshape(T, 128, D)
        vv_all[i] = (
            np.concatenate([v0, v1], axis=2).transpose(1, 0, 2).reshape(128, -1)
        )

    out_all = runner(qt_all, kt_all, vv_all)  # [NU, 128, n_ctx] = out^T

    out = np.empty((B, H, n_ctx, D), np.float32)
    for i, (bq, h, b0, b1) in enumerate(unit_specs):
        out[b0, h] = out_all[i, 0:64, :].T
        if b1 != b0:
            out[b1, h] = out_all[i, 64:128, :].T
    return out


def kernel(q, k, v, swap):
    q = np.ascontiguousarray(np.asarray(q, dtype=np.float32))
    k = np.ascontiguousarray(np.asarray(k, dtype=np.float32))
    v = np.ascontiguousarray(np.asarray(v, dtype=np.float32))
    swap_val = int(np.asarray(swap).reshape(-1)[0])

    n_ctx = q.shape[2]
    if swap_val:
        # 32 pair-units: attn of (b, h) applied to v[b] and v[b + B//2]
        specs = [(bq, h, bq, bq + B // 2) for bq in range(B // 2) for h in range(H)]
    else:
        # 64 independent units (2nd v slot duplicates the 1st)
        specs = [(b, h, b, b) for b in range(B) for h in range(H)]
    return _run_units(specs, q, k, v, n_ctx)


if __name__ == "__main__":
    rng = np.random.default_rng(0)
    q = rng.standard_normal((B, H, N, D), dtype=np.float32)
    k = rng.standard_normal((B, H, N, D), dtype=np.float32)
    v = rng.standard_normal((B, H, N, D), dtype=np.float32)
    o = kernel(q, k, v, 1)
    print("out", o.shape, o.dtype, float(np.abs(o).mean()))


# revision 22
# speedup vs baseline: 320.6670x; 1.0019x over previous
"""Trainium2 Bass kernel for nn_Attend_584115552611.

Attention B=4, H=16, N=2048, D=64 fp32 with the "swap" quirk:
when swap is truthy, attn probs of batches 0,1 are reused for batches 2,3
(each batch keeps its own v).  We therefore compute one softmax(QK^T) per
(qk-batch, head) "pair-unit" and apply it to two v tensors at once by
packing [v_b | v_{b+2}] into the 128 stationary PE columns.

Sharding: 32 pair-units (2 qk-batches x 16 heads) spread over 8 cores,
4 units per core (data/head parallel, no collectives).
"""

import sys
import functools

import numpy as np

for _p in ("/opt/trn_rl_repo",):
    if _p not in sys.path:
        sys.path.insert(0, _p)

import bass_rust
import concourse.bass as bass
import concourse.tile as tile
from concourse import mybir
from concourse.masks import make_identity

B, H, N, D = 4, 16, 2048, 64
N_CORES = 8
FP32 = mybir.dt.float32
BF16 = mybir.dt.bfloat16
FP32R = mybir.dt.float32r


def _split_excess_waits(nc, maxw=1):
    """This walrus build rejects instructions carrying more than one sync
    wait: spread excess waits onto inserted same-engine NOPs just before
    the offending instruction (engine queues are in-order, so semantics
    are unchanged)."""
    nid = 0
    for f in nc.m.functions:
        for bb in f.blocks:
            out = []
            changed = False
            for inst in bb.instructions:
                si = inst.sync_info
                waits = list(si.on_wait) if si and si.on_wait else []
                if len(waits) > maxw:
                    changed = True
                    for w in waits[:-maxw]:
                        nid += 1
                        nop = mybir.InstNoOp(name=f"I-waitsplit-{nid}")
                        nop.engine = inst.engine
                        nop.sync_info = bass_rust.SyncInfo(on_wait=[w], on_update=[])
                        out.append(nop)
                    si.on_wait = waits[-maxw:]
                out.append(inst)
            if changed:
                bb.instructions = out


def build_attn_program(n_units, n_ctx=N, d=D):
    """One softmax(q k^T * d^-0.5) per unit applied to TWO v tensors.

    The host pre-packs all operands into PE-friendly layouts, so the kernel
    does no transposes at all:
      qt  [U, 128, T*128]    f32  rows 0:64 = q^T (d-major), rows 64:128 = copy
                                  (replica feeds PE row-group 1)
      kt  [U, 128, T/2*128]  f32  rows 0:64 = k^T of even 128-row tiles,
                                  rows 64:128 = odd tiles (row-group packing)
      vv  [U, 128, T*128]    f32  [p, t*128 + w*64+dd] = v_w[t*128+p, dd]
      out [U, 128, n_ctx]    f32  out^T: [w*64+dd, q] (host transposes back)
    """
    assert d == 64 and n_ctx % 512 == 0
    T = n_ctx // 128          # k/q tiles of 128 rows
    NCH = n_ctx // 512        # 512-wide q chunks
    scale = float(d) ** -0.5

    nc = bass.Bass()
    qt = nc.declare_dram_parameter("qt", [n_units, 128, T * 128], FP32R, isOutput=False)
    kt = nc.declare_dram_parameter(
        "kt", [n_units, 128, (T // 2) * 128], FP32R, isOutput=False
    )
    vv = nc.declare_dram_parameter("vv", [n_units, 128, T * 128], BF16, isOutput=False)
    out = nc.declare_dram_parameter("out", [n_units, 128, n_ctx], FP32, isOutput=True)

    with tile.TileContext(nc) as tc:
        with (
            tc.tile_pool(name="singles", bufs=1) as singles,
            tc.tile_pool(name="ins", bufs=2) as ins_pool,
            tc.tile_pool(name="pt", bufs=2) as pt_pool,
            tc.tile_pool(name="sig", bufs=4) as sig_pool,
            tc.tile_pool(name="outs", bufs=2) as outs_pool,
            tc.tile_pool(name="qk_ps", bufs=3, space="PSUM") as qk_ps_pool,
            tc.tile_pool(name="av_ps", bufs=1, space="PSUM") as av_ps_pool,
            tc.tile_pool(name="sum_ps", bufs=1, space="PSUM") as sum_ps_pool,
        ):
            ones_bf = singles.tile([128, 128], BF16)
            nc.vector.memset(ones_bf, 1.0)

            for u in range(n_units):
                # ---- loads (contiguous; SWDGE DMA casts fp32 -> fp32r/bf16) ----
                qT_rep = ins_pool.tile([128, T, 128], FP32R, tag="qT")
                kT_st = ins_pool.tile([128, T // 2, 128], FP32R, tag="kT")
                vv_sb = ins_pool.tile([128, T, 128], BF16, tag="vv_sb")
                nc.sync.dma_start(
                    out=qT_rep, in_=qt[u].rearrange("p (t r) -> p t r", t=T)
                )
                nc.sync.dma_start(
                    out=kT_st, in_=kt[u].rearrange("p (j r) -> p j r", j=T // 2)
                )
                nc.sync.dma_start(
                    out=vv_sb, in_=vv[u].rearrange("p (t r) -> p t r", t=T)
                )

                # ---- main loop: software-pipelined over 512-wide q chunks.
                # Emit QK+exp for chunk c BEFORE AV/sums for chunk c-1 so
                # ScalarE exps chunk c while the PE streams chunk c-1's
                # AV+sums (otherwise the two phases serialize).
                oT_unit = outs_pool.tile([128, NCH, 512], FP32, tag="oT_unit")
                pT_tiles = {}
                for c in range(NCH + 1):
                    if c < NCH:
                        qs = c * 4  # first q-tile of this chunk
                        # P^T for this chunk: [k-in-tile, k-tile, q-in-chunk]
                        pT = pt_pool.tile([128, T, 512], BF16, tag="pT")
                        pT_tiles[c] = pT
                        for j in range(T // 2):
                            ps = qk_ps_pool.tile([128, 1024], FP32, tag="qk")
                            # row-group 0: k-tile 2j ; row-group 1: k-tile 2j+1
                            nc.tensor.matmul(
                                ps[:, 0:512],
                                lhsT=kT_st[0:64, j, :],
                                rhs=qT_rep[0:64, qs : qs + 4, :],
                                start=True,
                                stop=True,
                            )
                            nc.tensor.matmul(
                                ps[:, 512:1024],
                                lhsT=kT_st[64:128, j, :],
                                rhs=qT_rep[64:128, qs : qs + 4, :],
                                start=True,
                                stop=True,
                            )
                            # exp(scale * scores) for both k-tiles at once
                            nc.scalar.activation(
                                out=pT[:, 2 * j : 2 * j + 2, :],
                                in_=ps,
                                func=mybir.ActivationFunctionType.Exp,
                                scale=scale,
                            )

                    if c == 0:
                        continue
                    cc = c - 1
                    pT = pT_tiles.pop(cc)

                    # ---- AV (v-pair stationary) + column sums (ones) ----
                    av = av_ps_pool.tile([128, 512], FP32, tag="av")
                    # ones[128,128] stationary -> sigma replicated on all partitions
                    sm = sum_ps_pool.tile([128, 512], FP32, tag="sm")
                    for t in range(T):
                        nc.tensor.matmul(
                            av,
                            lhsT=vv_sb[:, t, :],
                            rhs=pT[:, t, :],
                            start=(t == 0),
                            stop=(t == T - 1),
                        )
                        nc.tensor.matmul(
                            sm,
                            lhsT=ones_bf,
                            rhs=pT[:, t, :],
                            start=(t == 0),
                            stop=(t == T - 1),
                        )

                    # ---- sigma -> reciprocal -> normalize (out^T stays) ----
                    rec_bc = sig_pool.tile([128, 512], FP32, tag="rec_bc")
                    nc.vector.reciprocal(out=rec_bc, in_=sm)
                    nc.vector.tensor_mul(oT_unit[:, cc, :], av, rec_bc)

                nc.sync.dma_start(
                    out=out[u].rearrange("p (c q) -> p c q", c=NCH), in_=oT_unit
                )

    _split_excess_waits(nc)
    return nc


@functools.lru_cache(maxsize=4)
def _get_program(n_units, n_ctx):
    return build_attn_program(n_units, n_ctx)


def _get_runner(n_units, n_ctx):
    """Build the bass program once and return a cached jitted SPMD runner:
    runner(concat_qk, concat_vv) -> concat_out  (axis 0 = cores*units)."""
    import jax
    from jax.experimental.shard_map import shard_map
    from jax.sharding import Mesh, PartitionSpec
    from concourse import bass2jax

    try:
        jax.config.update("jax_compilation_cache_dir", "/tmp/jax_neff_cache")
        jax.config.update("jax_persistent_cache_min_compile_time_secs", 10)
    except Exception:
        pass
    bass2jax.install_neuronx_cc_hook()
    nc = _get_program(n_units, n_ctx)

    in_names, out_names, out_avals, zero_shapes = [], [], [], []
    for alloc in nc.m.functions[0].allocations:
        if not isinstance(alloc, mybir.MemoryLocationSet):
            continue
        name = alloc.memorylocations[0].name
        if alloc.kind == "ExternalInput":
            if nc.partition_id_tensor is None or name != nc.partition_id_tensor.name:
                in_names.append(name)
        elif alloc.kind == "ExternalOutput":
            out_names.append(name)
            shape = tuple(alloc.tensor_shape)
            dtype = mybir.dt.np(alloc.dtype)
            out_avals.append(jax.core.ShapedArray(shape, dtype))
            zero_shapes.append((shape, dtype))
    assert in_names == ["qt", "kt", "vv"] and out_names == ["out"]
    n_params = len(in_names)
    all_names = in_names + out_names
    if nc.partition_id_tensor is not None:
        all_names.append(nc.partition_id_tensor.name)

    def _body(*args):
        operands = list(args)
        if nc.partition_id_tensor is not None:
            operands.append(bass2jax.partition_id_tensor())
        outs = bass2jax._bass_exec_p.bind(
            *operands,
            out_avals=tuple(out_avals),
            in_names=tuple(all_names),
            out_names=tuple(out_names),
            lowering_input_output_aliases=(),
            sim_require_finite=True,
            sim_require_nnan=True,
            nc=nc,
        )
        return tuple(outs)

    devices = jax.devices()[:N_CORES]
    mesh = Mesh(np.asarray(devices), ("core",))
    n_outs = len(out_names)
    sharded = jax.jit(
        shard_map(
            _body,
            mesh=mesh,
            in_specs=(PartitionSpec("core"),) * (n_params + n_outs),
            out_specs=(PartitionSpec("core"),) * n_outs,
            check_rep=False,
        ),
        keep_unused=True,
    )

    def runner(qt_all, kt_all, vv_all):
        zeros = [
            np.zeros((N_CORES * s[0], *s[1:]), dt) for (s, dt) in zero_shapes
        ]
        (out_all,) = sharded(qt_all, kt_all, vv_all, *zeros)
        return np.asarray(out_all)

    runner.sharded = sharded
    runner.mesh = mesh
    runner.zero_shapes = zero_shapes
    return runner


_RUNNERS = {}


def _run_units(unit_specs, q, k, v, n_ctx):
    """unit_specs: list of (qk_batch, head, v_batch0, v_batch1)."""
    n_units = len(unit_specs) // N_CORES
    assert n_units * N_CORES == len(unit_specs)
    key = (n_units, n_ctx)
    if key not in _RUNNERS:
        _RUNNERS[key] = _get_runner(n_units, n_ctx)
    runner = _RUNNERS[key]

    T = n_ctx // 128
    NU = N_CORES * n_units
    qt_all = np.empty((NU, 128, T * 128), np.float32)
    kt_all = np.empty((NU, 128, (T // 2) * 128), np.float32)
    import ml_dtypes
    vv_all = np.empty((NU, 128, T * 128), ml_dtypes.bfloat16)
    for i, (bq, h, b0, b1) in enumerate(unit_specs):
        qT = q[bq, h].T                      # [64, n_ctx], d-major
        qt_all[i, 0:64] = qT
        qt_all[i, 64:128] = qT               # replica feeds PE row-group 1
        kT = k[bq, h].T.reshape(64, T, 128)  # [dd, t, r]
        kt_all[i, 0:64] = kT[:, 0::2, :].reshape(64, -1)
        kt_all[i, 64:128] = kT[:, 1::2, :].reshape(64, -1)
        v0 = v[b0, h].reshape(T, 128, D)     # [t, p, dd]
        v1 = v[b1, h].reshape(T, 128, D)
        vv_all[i] = (
            np.concatenate([v0, v1], axis=2)
            .transpose(1, 0, 2)
            .reshape(128, -1)
            .astype(ml_dtypes.bfloat16)
        )

    out_all = runner(qt_all, kt_all, vv_all)  # [NU, 128, n_ctx] = out^T

    out = np.empty((B, H, n_ctx, D), np.float32)
    for i, (bq, h, b0, b1) in enumerate(unit_specs):
        out[b0, h] = out_all[i, 0:64, :].T
        if b1 != b0:
            out[b1, h] = out_all[i, 64:128, :].T
    return out


def kernel(q, k, v, swap):
    q = np.ascontiguousarray(np.asarray(q, dtype=np.float32))
    k = np.ascontiguousarray(np.asarray(k, dtype=np.float32))
    v = np.ascontiguousarray(np.asarray(v, dtype=np.float32))
    swap_val = int(np.asarray(swap).reshape(-1)[0])

    n_ctx = q.shape[2]
    if swap_val:
        # 32 pair-units: attn of (b, h) applied to v[b] and v[b + B//2]
        specs = [(bq, h, bq, bq + B // 2) for bq in range(B // 2) for h in range(H)]
    else:
        # 64 independent units (2nd v slot duplicates the 1st)
        specs = [(b, h, b, b) for b in range(B) for h in range(H)]
    return _run_units(specs, q, k, v, n_ctx)


if __name__ == "__main__":
    rng = np.random.default_rng(0)
    q = rng.standard_normal((B, H, N, D), dtype=np.float32)
    k = rng.standard_normal((B, H, N, D), dtype=np.float32)
    v = rng.standard_normal((B, H, N, D), dtype=np.float32)
    o = kernel(q, k, v, 1)
    print("out", o.shape, o.dtype, float(np.abs(o).mean()))


# revision 23
# speedup vs baseline: 323.4124x; 1.0086x over previous
"""Trainium2 Bass kernel for nn_Attend_584115552611.

Attention B=4, H=16, N=2048, D=64 fp32 with the "swap" quirk:
when swap is truthy, attn probs of batches 0,1 are reused for batches 2,3
(each batch keeps its own v).  We therefore compute one softmax(QK^T) per
(qk-batch, head) "pair-unit" and apply it to two v tensors at once by
packing [v_b | v_{b+2}] into the 128 stationary PE columns.

Sharding: 32 pair-units (2 qk-batches x 16 heads) spread over 8 cores,
4 units per core (data/head parallel, no collectives).
"""

import sys
import functools

import numpy as np

for _p in ("/opt/trn_rl_repo",):
    if _p not in sys.path:
        sys.path.insert(0, _p)

import bass_rust
import concourse.bass as bass
import concourse.tile as tile
from concourse import mybir
from concourse.masks import make_identity

B, H, N, D = 4, 16, 2048, 64
N_CORES = 8
FP32 = mybir.dt.float32
BF16 = mybir.dt.bfloat16
FP32R = mybir.dt.float32r


def _split_excess_waits(nc, maxw=1):
    """This walrus build rejects instructions carrying more than one sync
    wait: spread excess waits onto inserted same-engine NOPs just before
    the offending instruction (engine queues are in-order, so semantics
    are unchanged)."""
    nid = 0
    for f in nc.m.functions:
        for bb in f.blocks:
            out = []
            changed = False
            for inst in bb.instructions:
                si = inst.sync_info
                waits = list(si.on_wait) if si and si.on_wait else []
                if len(waits) > maxw:
                    changed = True
                    for w in waits[:-maxw]:
                        nid += 1
                        nop = mybir.InstNoOp(name=f"I-waitsplit-{nid}")
                        nop.engine = inst.engine
                        nop.sync_info = bass_rust.SyncInfo(on_wait=[w], on_update=[])
                        out.append(nop)
                    si.on_wait = waits[-maxw:]
                out.append(inst)
            if changed:
                bb.instructions = out


def build_attn_program(n_units, n_ctx=N, d=D):
    """One softmax(q k^T * d^-0.5) per unit applied to TWO v tensors.

    The host pre-packs all operands into PE-friendly layouts, so the kernel
    does no transposes at all:
      qt  [U, 128, T*128]    f32  rows 0:64 = q^T (d-major), rows 64:128 = copy
                                  (replica feeds PE row-group 1)
      kt  [U, 128, T/2*128]  f32  rows 0:64 = k^T of even 128-row tiles,
                                  rows 64:128 = odd tiles (row-group packing)
      vv  [U, 128, T*128]    f32  [p, t*128 + w*64+dd] = v_w[t*128+p, dd]
      out [U, 128, n_ctx]    f32  out^T: [w*64+dd, q] (host transposes back)
    """
    assert d == 64 and n_ctx % 512 == 0
    T = n_ctx // 128          # k/q tiles of 128 rows
    NCH = n_ctx // 512        # 512-wide q chunks
    scale = float(d) ** -0.5

    nc = bass.Bass()
    qt = nc.declare_dram_parameter("qt", [n_units, 128, T * 128], FP32R, isOutput=False)
    kt = nc.declare_dram_parameter(
        "kt", [n_units, 128, (T // 2) * 128], FP32R, isOutput=False
    )
    vv = nc.declare_dram_parameter("vv", [n_units, 128, T * 128], BF16, isOutput=False)
    out = nc.declare_dram_parameter("out", [n_units, 128, n_ctx], FP32, isOutput=True)

    with tile.TileContext(nc) as tc:
        with (
            tc.tile_pool(name="singles", bufs=1) as singles,
            tc.tile_pool(name="ins", bufs=2) as ins_pool,
            tc.tile_pool(name="pt", bufs=2) as pt_pool,
            tc.tile_pool(name="sig", bufs=4) as sig_pool,
            tc.tile_pool(name="outs", bufs=2) as outs_pool,
            tc.tile_pool(name="qk_ps", bufs=3, space="PSUM") as qk_ps_pool,
            tc.tile_pool(name="av_ps", bufs=1, space="PSUM") as av_ps_pool,
            tc.tile_pool(name="sum_ps", bufs=1, space="PSUM") as sum_ps_pool,
        ):
            ones_bf = singles.tile([128, 512], BF16)
            nc.vector.memset(ones_bf, 1.0)

            # Warm up the PE (HAM clock gate) while the first unit's DMA
            # loads are in flight: ~3.5us of dummy matmuls so real work
            # starts at 2.4 GHz instead of the cold 1.2 GHz.
            warm = av_ps_pool.tile([128, 512], FP32, tag="av")
            for _ in range(16):
                nc.tensor.matmul(
                    warm, lhsT=ones_bf[:, 0:128], rhs=ones_bf, start=True, stop=True
                )

            for u in range(n_units):
                # ---- loads (contiguous; SWDGE DMA casts fp32 -> fp32r/bf16) ----
                qT_rep = ins_pool.tile([128, T, 128], FP32R, tag="qT")
                kT_st = ins_pool.tile([128, T // 2, 128], FP32R, tag="kT")
                vv_sb = ins_pool.tile([128, T, 128], BF16, tag="vv_sb")
                nc.sync.dma_start(
                    out=qT_rep, in_=qt[u].rearrange("p (t r) -> p t r", t=T)
                )
                nc.sync.dma_start(
                    out=kT_st, in_=kt[u].rearrange("p (j r) -> p j r", j=T // 2)
                )
                nc.sync.dma_start(
                    out=vv_sb, in_=vv[u].rearrange("p (t r) -> p t r", t=T)
                )

                # ---- main loop: software-pipelined over 512-wide q chunks.
                # Emit QK+exp for chunk c BEFORE AV/sums for chunk c-1 so
                # ScalarE exps chunk c while the PE streams chunk c-1's
                # AV+sums (otherwise the two phases serialize).
                oT_unit = outs_pool.tile([128, NCH, 512], FP32, tag="oT_unit")
                pT_tiles = {}
                for c in range(NCH + 1):
                    if c < NCH:
                        qs = c * 4  # first q-tile of this chunk
                        # P^T for this chunk: [k-in-tile, k-tile, q-in-chunk]
                        pT = pt_pool.tile([128, T, 512], BF16, tag="pT")
                        pT_tiles[c] = pT
                        for j in range(T // 2):
                            ps = qk_ps_pool.tile([128, 1024], FP32, tag="qk")
                            # row-group 0: k-tile 2j ; row-group 1: k-tile 2j+1
                            nc.tensor.matmul(
                                ps[:, 0:512],
                                lhsT=kT_st[0:64, j, :],
                                rhs=qT_rep[0:64, qs : qs + 4, :],
                                start=True,
                                stop=True,
                            )
                            nc.tensor.matmul(
                                ps[:, 512:1024],
                                lhsT=kT_st[64:128, j, :],
                                rhs=qT_rep[64:128, qs : qs + 4, :],
                                start=True,
                                stop=True,
                            )
                            # exp(scale * scores) for both k-tiles at once
                            nc.scalar.activation(
                                out=pT[:, 2 * j : 2 * j + 2, :],
                                in_=ps,
                                func=mybir.ActivationFunctionType.Exp,
                                scale=scale,
                            )

                    if c == 0:
                        continue
                    cc = c - 1
                    pT = pT_tiles.pop(cc)

                    # ---- AV (v-pair stationary) + column sums (ones) ----
                    av = av_ps_pool.tile([128, 512], FP32, tag="av")
                    # ones[128,128] stationary -> sigma replicated on all partitions
                    sm = sum_ps_pool.tile([128, 512], FP32, tag="sm")
                    for t in range(T):
                        nc.tensor.matmul(
                            av,
                            lhsT=vv_sb[:, t, :],
                            rhs=pT[:, t, :],
                            start=(t == 0),
                            stop=(t == T - 1),
                        )
                        nc.tensor.matmul(
                            sm,
                            lhsT=ones_bf[:, 0:128],
                            rhs=pT[:, t, :],
                            start=(t == 0),
                            stop=(t == T - 1),
                        )

                    # ---- sigma -> reciprocal -> normalize (out^T stays) ----
                    rec_bc = sig_pool.tile([128, 512], FP32, tag="rec_bc")
                    nc.vector.reciprocal(out=rec_bc, in_=sm)
                    nc.vector.tensor_mul(oT_unit[:, cc, :], av, rec_bc)

                nc.sync.dma_start(
                    out=out[u].rearrange("p (c q) -> p c q", c=NCH), in_=oT_unit
                )

    _split_excess_waits(nc)
    return nc


@functools.lru_cache(maxsize=4)
def _get_program(n_units, n_ctx):
    return build_attn_program(n_units, n_ctx)


def _get_runner(n_units, n_ctx):
    """Build the bass program once and return a cached jitted SPMD runner:
    runner(concat_qk, concat_vv) -> concat_out  (axis 0 = cores*units)."""
    import jax
    from jax.experimental.shard_map import shard_map
    from jax.sharding import Mesh, PartitionSpec
    from concourse import bass2jax

    try:
        jax.config.update("jax_compilation_cache_dir", "/tmp/jax_neff_cache")
        jax.config.update("jax_persistent_cache_min_compile_time_secs", 10)
    except Exception:
        pass
    bass2jax.install_neuronx_cc_hook()
    nc = _get_program(n_units, n_ctx)

    in_names, out_names, out_avals, zero_shapes = [], [], [], []
    for alloc in nc.m.functions[0].allocations:
        if not isinstance(alloc, mybir.MemoryLocationSet):
            continue
        name = alloc.memorylocations[0].name
        if alloc.kind == "ExternalInput":
            if nc.partition_id_tensor is None or name != nc.partition_id_tensor.name:
                in_names.append(name)
        elif alloc.kind == "ExternalOutput":
            out_names.append(name)
            shape = tuple(alloc.tensor_shape)
            dtype = mybir.dt.np(alloc.dtype)
            out_avals.append(jax.core.ShapedArray(shape, dtype))
            zero_shapes.append((shape, dtype))
    assert in_names == ["qt", "kt", "vv"] and out_names == ["out"]
    n_params = len(in_names)
    all_names = in_names + out_names
    if nc.partition_id_tensor is not None:
        all_names.append(nc.partition_id_tensor.name)

    def _body(*args):
        operands = list(args)
        if nc.partition_id_tensor is not None:
            operands.append(bass2jax.partition_id_tensor())
        outs = bass2jax._bass_exec_p.bind(
            *operands,
            out_avals=tuple(out_avals),
            in_names=tuple(all_names),
            out_names=tuple(out_names),
            lowering_input_output_aliases=(),
            sim_require_finite=True,
            sim_require_nnan=True,
            nc=nc,
        )
        return tuple(outs)

    devices = jax.devices()[:N_CORES]
    mesh = Mesh(np.asarray(devices), ("core",))
    n_outs = len(out_names)
    sharded = jax.jit(
        shard_map(
            _body,
            mesh=mesh,
            in_specs=(PartitionSpec("core"),) * (n_params + n_outs),
            out_specs=(PartitionSpec("core"),) * n_outs,
            check_rep=False,
        ),
        keep_unused=True,
    )

    def runner(qt_all, kt_all, vv_all):
        zeros = [
            np.zeros((N_CORES * s[0], *s[1:]), dt) for (s, dt) in zero_shapes
        ]
        (out_all,) = sharded(qt_all, kt_all, vv_all, *zeros)
        return np.asarray(out_all)

    runner.sharded = sharded
    runner.mesh = mesh
    runner.zero_shapes = zero_shapes
    return runner


_RUNNERS = {}


def _run_units(unit_specs, q, k, v, n_ctx):
    """unit_specs: list of (qk_batch, head, v_batch0, v_batch1)."""
    n_units = len(unit_specs) // N_CORES
    assert n_units * N_CORES == len(unit_specs)
    key = (n_units, n_ctx)
    if key not in _RUNNERS:
        _RUNNERS[key] = _get_runner(n_units, n_ctx)
    runner = _RUNNERS[key]

    T = n_ctx // 128
    NU = N_CORES * n_units
    qt_all = np.empty((NU, 128, T * 128), np.float32)
    kt_all = np.empty((NU, 128, (T // 2) * 128), np.float32)
    import ml_dtypes
    vv_all = np.empty((NU, 128, T * 128), ml_dtypes.bfloat16)
    for i, (bq, h, b0, b1) in enumerate(unit_specs):
        qT = q[bq, h].T                      # [64, n_ctx], d-major
        qt_all[i, 0:64] = qT
        qt_all[i, 64:128] = qT               # replica feeds PE row-group 1
        kT = k[bq, h].T.reshape(64, T, 128)  # [dd, t, r]
        kt_all[i, 0:64] = kT[:, 0::2, :].reshape(64, -1)
        kt_all[i, 64:128] = kT[:, 1::2, :].reshape(64, -1)
        v0 = v[b0, h].reshape(T, 128, D)     # [t, p, dd]
        v1 = v[b1, h].reshape(T, 128, D)
        vv_all[i] = (
            np.concatenate([v0, v1], axis=2)
            .transpose(1, 0, 2)
            .reshape(128, -1)
            .astype(ml_dtypes.bfloat16)
        )

    out_all = runner(qt_all, kt_all, vv_all)  # [NU, 128, n_ctx] = out^T

    out = np.empty((B, H, n_ctx, D), np.float32)
    for i, (bq, h, b0, b1) in enumerate(unit_specs):
        out[b0, h] = out_all[i, 0:64, :].T
        if b1 != b0:
            out[b1, h] = out_all[i, 64:128, :].T
    return out


def kernel(q, k, v, swap):
    q = np.ascontiguousarray(np.asarray(q, dtype=np.float32))
    k = np.ascontiguousarray(np.asarray(k, dtype=np.float32))
    v = np.ascontiguousarray(np.asarray(v, dtype=np.float32))
    swap_val = int(np.asarray(swap).reshape(-1)[0])

    n_ctx = q.shape[2]
    if swap_val:
        # 32 pair-units: attn of (b, h) applied to v[b] and v[b + B//2]
        specs = [(bq, h, bq, bq + B // 2) for bq in range(B // 2) for h in range(H)]
    else:
        # 64 independent units (2nd v slot duplicates the 1st)
        specs = [(b, h, b, b) for b in range(B) for h in range(H)]
    return _run_units(specs, q, k, v, n_ctx)


if __name__ == "__main__":
    rng = np.random.default_rng(0)
    q = rng.standard_normal((B, H, N, D), dtype=np.float32)
    k = rng.standard_normal((B, H, N, D), dtype=np.float32)
    v = rng.standard_normal((B, H, N, D), dtype=np.float32)
    o = kernel(q, k, v, 1)
    print("out", o.shape, o.dtype, float(np.abs(o).mean()))


# revision 25
# speedup vs baseline: 325.2505x; 1.0057x over previous
"""Trainium2 Bass kernel for nn_Attend_584115552611.

Attention B=4, H=16, N=2048, D=64 fp32 with the "swap" quirk:
when swap is truthy, attn probs of batches 0,1 are reused for batches 2,3
(each batch keeps its own v).  We therefore compute one softmax(QK^T) per
(qk-batch, head) "pair-unit" and apply it to two v tensors at once by
packing [v_b | v_{b+2}] into the 128 stationary PE columns.

Sharding: 32 pair-units (2 qk-batches x 16 heads) spread over 8 cores,
4 units per core (data/head parallel, no collectives).
"""

import sys
import functools

import numpy as np

for _p in ("/opt/trn_rl_repo",):
    if _p not in sys.path:
        sys.path.insert(0, _p)

import bass_rust
import concourse.bass as bass
import concourse.tile as tile
from concourse import mybir
from concourse.masks import make_identity

B, H, N, D = 4, 16, 2048, 64
N_CORES = 8
FP32 = mybir.dt.float32
BF16 = mybir.dt.bfloat16
FP32R = mybir.dt.float32r


def _split_excess_waits(nc, maxw=1):
    """This walrus build rejects instructions carrying more than one sync
    wait: spread excess waits onto inserted same-engine NOPs just before
    the offending instruction (engine queues are in-order, so semantics
    are unchanged)."""
    nid = 0
    for f in nc.m.functions:
        for bb in f.blocks:
            out = []
            changed = False
            for inst in bb.instructions:
                si = inst.sync_info
                waits = list(si.on_wait) if si and si.on_wait else []
                if len(waits) > maxw:
                    changed = True
                    for w in waits[:-maxw]:
                        nid += 1
                        nop = mybir.InstNoOp(name=f"I-waitsplit-{nid}")
                        nop.engine = inst.engine
                        nop.sync_info = bass_rust.SyncInfo(on_wait=[w], on_update=[])
                        out.append(nop)
                    si.on_wait = waits[-maxw:]
                out.append(inst)
            if changed:
                bb.instructions = out


def build_attn_program(n_units, n_ctx=N, d=D):
    """One softmax(q k^T * d^-0.5) per unit applied to TWO v tensors.

    The host pre-packs all operands into PE-friendly layouts, so the kernel
    does no transposes at all:
      qt  [U, 128, T*128]    f32  rows 0:64 = q^T (d-major), rows 64:128 = copy
                                  (replica feeds PE row-group 1)
      kt  [U, 128, T/2*128]  f32  rows 0:64 = k^T of even 128-row tiles,
                                  rows 64:128 = odd tiles (row-group packing)
      vv  [U, 128, T*128]    f32  [p, t*128 + w*64+dd] = v_w[t*128+p, dd]
      out [U, 128, n_ctx]    f32  out^T: [w*64+dd, q] (host transposes back)
    """
    assert d == 64 and n_ctx % 512 == 0
    T = n_ctx // 128          # k/q tiles of 128 rows
    NCH = n_ctx // 512        # 512-wide q chunks
    scale = float(d) ** -0.5

    nc = bass.Bass()
    qt = nc.declare_dram_parameter("qt", [n_units, 128, T * 128], FP32R, isOutput=False)
    kt = nc.declare_dram_parameter(
        "kt", [n_units, 128, (T // 2) * 128], FP32R, isOutput=False
    )
    vv = nc.declare_dram_parameter("vv", [n_units, 128, T * 128], BF16, isOutput=False)
    out = nc.declare_dram_parameter("out", [n_units, 128, n_ctx], FP32, isOutput=True)

    with tile.TileContext(nc) as tc:
        with (
            tc.tile_pool(name="singles", bufs=1) as singles,
            tc.tile_pool(name="ins", bufs=2) as ins_pool,
            tc.tile_pool(name="pt", bufs=2) as pt_pool,
            tc.tile_pool(name="sig", bufs=4) as sig_pool,
            tc.tile_pool(name="outs", bufs=2) as outs_pool,
            tc.tile_pool(name="qk_ps", bufs=3, space="PSUM") as qk_ps_pool,
            tc.tile_pool(name="av_ps", bufs=1, space="PSUM") as av_ps_pool,
            tc.tile_pool(name="sum_ps", bufs=1, space="PSUM") as sum_ps_pool,
        ):
            ones_bf = singles.tile([128, 512], BF16)
            nc.vector.memset(ones_bf, 1.0)

            # Warm up the PE (HAM clock gate) while the first unit's DMA
            # loads are in flight: ~3.5us of dummy matmuls so real work
            # starts at 2.4 GHz instead of the cold 1.2 GHz.
            warm = av_ps_pool.tile([128, 512], FP32, tag="av")
            for _ in range(16):
                nc.tensor.matmul(
                    warm, lhsT=ones_bf[:, 0:128], rhs=ones_bf, start=True, stop=True
                )

            # ---- flat software pipeline over all (unit, chunk) slots:
            # emit QK+exp for slot i and AV/sums for slot i-1, ACROSS unit
            # boundaries, so neither PE nor ScalarE bubbles between units.
            slots = [(u, c) for u in range(n_units) for c in range(NCH)]
            ins_tiles = {}
            oT_units = {}
            pT_tiles = {}
            for i in range(len(slots) + 1):
                if i < len(slots):
                    u, c = slots[i]
                    if c == 0:
                        # loads (contiguous; HWDGE) + per-unit output tile
                        qT_rep = ins_pool.tile([128, T, 128], FP32R, tag="qT")
                        kT_st = ins_pool.tile([128, T // 2, 128], FP32R, tag="kT")
                        vv_sb = ins_pool.tile([128, T, 128], BF16, tag="vv_sb")
                        nc.sync.dma_start(
                            out=qT_rep, in_=qt[u].rearrange("p (t r) -> p t r", t=T)
                        )
                        nc.sync.dma_start(
                            out=kT_st,
                            in_=kt[u].rearrange("p (j r) -> p j r", j=T // 2),
                        )
                        nc.sync.dma_start(
                            out=vv_sb, in_=vv[u].rearrange("p (t r) -> p t r", t=T)
                        )
                        ins_tiles[u] = (qT_rep, kT_st, vv_sb)
                        oT_unit = outs_pool.tile([128, NCH, 512], FP32, tag="oT_unit")
                        oT_units[u] = oT_unit
                    qT_rep, kT_st, vv_sb = ins_tiles[u]
                    qs = c * 4  # first q-tile of this chunk
                    # P^T for this chunk: [k-in-tile, k-tile, q-in-chunk]
                    pT = pt_pool.tile([128, T, 512], BF16, tag="pT")
                    pT_tiles[u, c] = pT
                    for j in range(T // 2):
                        ps = qk_ps_pool.tile([128, 1024], FP32, tag="qk")
                        # row-group 0: k-tile 2j ; row-group 1: k-tile 2j+1
                        nc.tensor.matmul(
                            ps[:, 0:512],
                            lhsT=kT_st[0:64, j, :],
                            rhs=qT_rep[0:64, qs : qs + 4, :],
                            start=True,
                            stop=True,
                        )
                        nc.tensor.matmul(
                            ps[:, 512:1024],
                            lhsT=kT_st[64:128, j, :],
                            rhs=qT_rep[64:128, qs : qs + 4, :],
                            start=True,
                            stop=True,
                        )
                        # exp(scale * scores) for both k-tiles at once
                        nc.scalar.activation(
                            out=pT[:, 2 * j : 2 * j + 2, :],
                            in_=ps,
                            func=mybir.ActivationFunctionType.Exp,
                            scale=scale,
                        )

                if i == 0:
                    continue
                u2, c2 = slots[i - 1]
                _, _, vv2 = ins_tiles[u2]
                pT = pT_tiles.pop((u2, c2))

                # ---- AV (v-pair stationary) + column sums (ones) ----
                av = av_ps_pool.tile([128, 512], FP32, tag="av")
                # ones[128,128] stationary -> sigma replicated on all partitions
                sm = sum_ps_pool.tile([128, 512], FP32, tag="sm")
                for t in range(T):
                    nc.tensor.matmul(
                        av,
                        lhsT=vv2[:, t, :],
                        rhs=pT[:, t, :],
                        start=(t == 0),
                        stop=(t == T - 1),
                    )
                    nc.tensor.matmul(
                        sm,
                        lhsT=ones_bf[:, 0:128],
                        rhs=pT[:, t, :],
                        start=(t == 0),
                        stop=(t == T - 1),
                    )

                # ---- sigma -> reciprocal -> normalize (out^T stays) ----
                rec_bc = sig_pool.tile([128, 512], FP32, tag="rec_bc")
                nc.vector.reciprocal(out=rec_bc, in_=sm)
                nc.vector.tensor_mul(oT_units[u2][:, c2, :], av, rec_bc)

                if c2 == NCH - 1:
                    nc.sync.dma_start(
                        out=out[u2].rearrange("p (c q) -> p c q", c=NCH),
                        in_=oT_units.pop(u2),
                    )
                    ins_tiles.pop(u2)

    _split_excess_waits(nc)
    return nc


@functools.lru_cache(maxsize=4)
def _get_program(n_units, n_ctx):
    return build_attn_program(n_units, n_ctx)


def _get_runner(n_units, n_ctx):
    """Build the bass program once and return a cached jitted SPMD runner:
    runner(concat_qk, concat_vv) -> concat_out  (axis 0 = cores*units)."""
    import jax
    from jax.experimental.shard_map import shard_map
    from jax.sharding import Mesh, PartitionSpec
    from concourse import bass2jax

    try:
        jax.config.update("jax_compilation_cache_dir", "/tmp/jax_neff_cache")
        jax.config.update("jax_persistent_cache_min_compile_time_secs", 10)
    except Exception:
        pass
    bass2jax.install_neuronx_cc_hook()
    nc = _get_program(n_units, n_ctx)

    in_names, out_names, out_avals, zero_shapes = [], [], [], []
    for alloc in nc.m.functions[0].allocations:
        if not isinstance(alloc, mybir.MemoryLocationSet):
            continue
        name = alloc.memorylocations[0].name
        if alloc.kind == "ExternalInput":
            if nc.partition_id_tensor is None or name != nc.partition_id_tensor.name:
                in_names.append(name)
        elif alloc.kind == "ExternalOutput":
            out_names.append(name)
            shape = tuple(alloc.tensor_shape)
            dtype = mybir.dt.np(alloc.dtype)
            out_avals.append(jax.core.ShapedArray(shape, dtype))
            zero_shapes.append((shape, dtype))
    assert in_names == ["qt", "kt", "vv"] and out_names == ["out"]
    n_params = len(in_names)
    all_names = in_names + out_names
    if nc.partition_id_tensor is not None:
        all_names.append(nc.partition_id_tensor.name)

    def _body(*args):
        operands = list(args)
        if nc.partition_id_tensor is not None:
            operands.append(bass2jax.partition_id_tensor())
        outs = bass2jax._bass_exec_p.bind(
            *operands,
            out_avals=tuple(out_avals),
            in_names=tuple(all_names),
            out_names=tuple(out_names),
            lowering_input_output_aliases=(),
            sim_require_finite=True,
            sim_require_nnan=True,
            nc=nc,
        )
        return tuple(outs)

    devices = jax.devices()[:N_CORES]
    mesh = Mesh(np.asarray(devices), ("core",))
    n_outs = len(out_names)
    sharded = jax.jit(
        shard_map(
            _body,
            mesh=mesh,
            in_specs=(PartitionSpec("core"),) * (n_params + n_outs),
            out_specs=(PartitionSpec("core"),) * n_outs,
            check_rep=False,
        ),
        keep_unused=True,
    )

    def runner(qt_all, kt_all, vv_all):
        zeros = [
            np.zeros((N_CORES * s[0], *s[1:]), dt) for (s, dt) in zero_shapes
        ]
        (out_all,) = sharded(qt_all, kt_all, vv_all, *zeros)
        return np.asarray(out_all)

    runner.sharded = sharded
    runner.mesh = mesh
    runner.zero_shapes = zero_shapes
    return runner


_RUNNERS = {}


def _run_units(unit_specs, q, k, v, n_ctx):
    """unit_specs: list of (qk_batch, head, v_batch0, v_batch1)."""
    n_units = len(unit_specs) // N_CORES
    assert n_units * N_CORES == len(unit_specs)
    key = (n_units, n_ctx)
    if key not in _RUNNERS:
        _RUNNERS[key] = _get_runner(n_units, n_ctx)
    runner = _RUNNERS[key]

    T = n_ctx // 128
    NU = N_CORES * n_units
    qt_all = np.empty((NU, 128, T * 128), np.float32)
    kt_all = np.empty((NU, 128, (T // 2) * 128), np.float32)
    import ml_dtypes
    vv_all = np.empty((NU, 128, T * 128), ml_dtypes.bfloat16)
    for i, (bq, h, b0, b1) in enumerate(unit_specs):
        qT = q[bq, h].T                      # [64, n_ctx], d-major
        qt_all[i, 0:64] = qT
        qt_all[i, 64:128] = qT               # replica feeds PE row-group 1
        kT = k[bq, h].T.reshape(64, T, 128)  # [dd, t, r]
        kt_all[i, 0:64] = kT[:, 0::2, :].reshape(64, -1)
        kt_all[i, 64:128] = kT[:, 1::2, :].reshape(64, -1)
        v0 = v[b0, h].reshape(T, 128, D)     # [t, p, dd]
        v1 = v[b1, h].reshape(T, 128, D)
        vv_all[i] = (
            np.concatenate([v0, v1], axis=2)
            .transpose(1, 0, 2)
            .reshape(128, -1)
            .astype(ml_dtypes.bfloat16)
        )

    out_all = runner(qt_all, kt_all, vv_all)  # [NU, 128, n_ctx] = out^T

    out = np.empty((B, H, n_ctx, D), np.float32)
    for i, (bq, h, b0, b1) in enumerate(unit_specs):
        out[b0, h] = out_all[i, 0:64, :].T
        if b1 != b0:
            out[b1, h] = out_all[i, 64:128, :].T
    return out


def kernel(q, k, v, swap):
    q = np.ascontiguousarray(np.asarray(q, dtype=np.float32))
    k = np.ascontiguousarray(np.asarray(k, dtype=np.float32))
    v = np.ascontiguousarray(np.asarray(v, dtype=np.float32))
    swap_val = int(np.asarray(swap).reshape(-1)[0])

    n_ctx = q.shape[2]
    if swap_val:
        # 32 pair-units: attn of (b, h) applied to v[b] and v[b + B//2]
        specs = [(bq, h, bq, bq + B // 2) for bq in range(B // 2) for h in range(H)]
    else:
        # 64 independent units (2nd v slot duplicates the 1st)
        specs = [(b, h, b, b) for b in range(B) for h in range(H)]
    return _run_units(specs, q, k, v, n_ctx)


if __name__ == "__main__":
    rng = np.random.default_rng(0)
    q = rng.standard_normal((B, H, N, D), dtype=np.float32)
    k = rng.standard_normal((B, H, N, D), dtype=np.float32)
    v = rng.standard_normal((B, H, N, D), dtype=np.float32)
    o = kernel(q, k, v, 1)
    print("out", o.shape, o.dtype, float(np.abs(o).mean()))


# revision 30
# speedup vs baseline: 331.8730x; 1.0204x over previous
"""Trainium2 Bass kernel for nn_Attend_584115552611.

Attention B=4, H=16, N=2048, D=64 fp32 with the "swap" quirk:
when swap is truthy, attn probs of batches 0,1 are reused for batches 2,3
(each batch keeps its own v).  We therefore compute one softmax(QK^T) per
(qk-batch, head) "pair-unit" and apply it to two v tensors at once by
packing [v_b | v_{b+2}] into the 128 stationary PE columns.

Sharding: 32 pair-units (2 qk-batches x 16 heads) spread over 8 cores,
4 units per core (data/head parallel, no collectives).
"""

import sys
import functools

import numpy as np

for _p in ("/opt/trn_rl_repo",):
    if _p not in sys.path:
        sys.path.insert(0, _p)

import bass_rust
import concourse.bass as bass
import concourse.tile as tile
from concourse import mybir
from concourse.masks import make_identity

B, H, N, D = 4, 16, 2048, 64
N_CORES = 8
FP32 = mybir.dt.float32
BF16 = mybir.dt.bfloat16
FP32R = mybir.dt.float32r


def _split_excess_waits(nc, maxw=1):
    """This walrus build rejects instructions carrying more than one sync
    wait: spread excess waits onto inserted same-engine NOPs just before
    the offending instruction (engine queues are in-order, so semantics
    are unchanged)."""
    nid = 0
    for f in nc.m.functions:
        for bb in f.blocks:
            out = []
            changed = False
            for inst in bb.instructions:
                si = inst.sync_info
                waits = list(si.on_wait) if si and si.on_wait else []
                if len(waits) > maxw:
                    changed = True
                    for w in waits[:-maxw]:
                        nid += 1
                        nop = mybir.InstNoOp(name=f"I-waitsplit-{nid}")
                        nop.engine = inst.engine
                        nop.sync_info = bass_rust.SyncInfo(on_wait=[w], on_update=[])
                        out.append(nop)
                    si.on_wait = waits[-maxw:]
                out.append(inst)
            if changed:
                bb.instructions = out


def build_attn_program(n_units, n_ctx=N, d=D):
    """One softmax(q k^T * d^-0.5) per unit applied to TWO v tensors.

    The host pre-packs all operands into PE-friendly layouts, so the kernel
    does no transposes at all:
      qt  [U, 128, T*128]    f32  rows 0:64 = q^T (d-major), rows 64:128 = copy
                                  (replica feeds PE row-group 1)
      kt  [U, 128, T/2*128]  f32  rows 0:64 = k^T of even 128-row tiles,
                                  rows 64:128 = odd tiles (row-group packing)
      vv  [U, 128, T*128]    f32  [p, t*128 + w*64+dd] = v_w[t*128+p, dd]
      out [U, 128, n_ctx]    f32  out^T: [w*64+dd, q] (host transposes back)
    """
    assert d == 64 and n_ctx % 512 == 0
    T = n_ctx // 128          # k/q tiles of 128 rows
    NCH = n_ctx // 512        # 512-wide q chunks
    scale = float(d) ** -0.5

    nc = bass.Bass()
    qt = nc.declare_dram_parameter("qt", [n_units, 128, T * 128], FP32R, isOutput=False)
    kt = nc.declare_dram_parameter(
        "kt", [n_units, 128, (T // 2) * 128], FP32R, isOutput=False
    )
    vv = nc.declare_dram_parameter("vv", [n_units, 128, T * 128], BF16, isOutput=False)
    out = nc.declare_dram_parameter("out", [n_units, 128, n_ctx], FP32, isOutput=True)

    with tile.TileContext(nc) as tc:
        with (
            tc.tile_pool(name="singles", bufs=1) as singles,
            tc.tile_pool(name="ins", bufs=3) as ins_pool,
            tc.tile_pool(name="pt", bufs=3) as pt_pool,
            tc.tile_pool(name="sig", bufs=4) as sig_pool,
            tc.tile_pool(name="outs", bufs=2) as outs_pool,
            tc.tile_pool(name="qk_ps", bufs=2, space="PSUM") as qk_ps_pool,
            tc.tile_pool(name="av_ps", bufs=2, space="PSUM") as av_ps_pool,
            tc.tile_pool(name="sum_ps", bufs=2, space="PSUM") as sum_ps_pool,
        ):
            ones_bf = singles.tile([128, 512], BF16)
            nc.vector.memset(ones_bf, 1.0)

            # Warm up the PE (HAM clock gate) while the first unit's DMA
            # loads are in flight: ~3.5us of dummy matmuls so real work
            # starts at 2.4 GHz instead of the cold 1.2 GHz.
            warm = av_ps_pool.tile([128, 512], FP32, tag="av")
            for _ in range(16):
                nc.tensor.matmul(
                    warm, lhsT=ones_bf[:, 0:128], rhs=ones_bf, start=True, stop=True
                )

            # ---- flat software pipeline over all (unit, chunk) slots:
            # emit QK+exp for slot i and AV/sums for slot i-1, ACROSS unit
            # boundaries, so neither PE nor ScalarE bubbles between units.
            slots = [(u, c) for u in range(n_units) for c in range(NCH)]
            ins_tiles = {}
            oT_units = {}
            pT_tiles = {}
            for i in range(len(slots) + 1):
                if i < len(slots):
                    u, c = slots[i]
                    if c == 0:
                        # loads (contiguous; HWDGE) + per-unit output tile
                        qT_rep = ins_pool.tile([128, T, 128], FP32R, tag="qT")
                        kT_st = ins_pool.tile([128, T // 2, 128], FP32R, tag="kT")
                        vv_sb = ins_pool.tile([128, T, 128], BF16, tag="vv_sb")
                        nc.sync.dma_start(
                            out=qT_rep, in_=qt[u].rearrange("p (t r) -> p t r", t=T)
                        )
                        nc.sync.dma_start(
                            out=kT_st,
                            in_=kt[u].rearrange("p (j r) -> p j r", j=T // 2),
                        )
                        nc.sync.dma_start(
                            out=vv_sb, in_=vv[u].rearrange("p (t r) -> p t r", t=T)
                        )
                        ins_tiles[u] = (qT_rep, kT_st, vv_sb)
                        oT_unit = outs_pool.tile([128, NCH, 512], FP32, tag="oT_unit")
                        oT_units[u] = oT_unit
                    qT_rep, kT_st, vv_sb = ins_tiles[u]
                    qs = c * 4  # first q-tile of this chunk
                    # P^T for this chunk: [k-in-tile, k-tile, q-in-chunk]
                    pT = pt_pool.tile([128, T, 512], BF16, tag="pT")
                    pT_tiles[u, c] = pT
                    for j in range(T // 2):
                        ps = qk_ps_pool.tile([128, 1024], FP32, tag="qk")
                        # row-group 0: k-tile 2j ; row-group 1: k-tile 2j+1
                        nc.tensor.matmul(
                            ps[:, 0:512],
                            lhsT=kT_st[0:64, j, :],
                            rhs=qT_rep[0:64, qs : qs + 4, :],
                            start=True,
                            stop=True,
                        )
                        nc.tensor.matmul(
                            ps[:, 512:1024],
                            lhsT=kT_st[64:128, j, :],
                            rhs=qT_rep[64:128, qs : qs + 4, :],
                            start=True,
                            stop=True,
                        )
                        # exp(scale * scores) for both k-tiles at once
                        nc.scalar.activation(
                            out=pT[:, 2 * j : 2 * j + 2, :],
                            in_=ps,
                            func=mybir.ActivationFunctionType.Exp,
                            scale=scale,
                        )

                if i == 0:
                    continue
                u2, c2 = slots[i - 1]
                _, _, vv2 = ins_tiles[u2]
                pT = pT_tiles.pop((u2, c2))

                # ---- AV (v-pair stationary) + column sums (ones) ----
                av = av_ps_pool.tile([128, 512], FP32, tag="av")
                # ones[128,128] stationary -> sigma replicated on all partitions
                sm = sum_ps_pool.tile([128, 512], FP32, tag="sm")
                # sums first so the reciprocal (DVE) overlaps the AV stream
                for t in range(T):
                    nc.tensor.matmul(
                        sm,
                        lhsT=ones_bf[:, 0:128],
                        rhs=pT[:, t, :],
                        start=(t == 0),
                        stop=(t == T - 1),
                    )
                for t in range(T):
                    nc.tensor.matmul(
                        av,
                        lhsT=vv2[:, t, :],
                        rhs=pT[:, t, :],
                        start=(t == 0),
                        stop=(t == T - 1),
                    )

                # ---- sigma -> reciprocal -> normalize (out^T stays) ----
                rec_bc = sig_pool.tile([128, 512], FP32, tag="rec_bc")
                nc.vector.reciprocal(out=rec_bc, in_=sm)
                oT = oT_units[u2]
                nc.vector.tensor_mul(oT[:, c2, :], av, rec_bc)
                # stream each chunk out as soon as it's normalized, so the
                # kernel tail only waits on the final 256KB store
                nc.sync.dma_start(
                    out=out[u2, :, c2 * 512 : (c2 + 1) * 512], in_=oT[:, c2, :]
                )
                if c2 == NCH - 1:
                    oT_units.pop(u2)
                    ins_tiles.pop(u2)

    _split_excess_waits(nc)
    return nc


@functools.lru_cache(maxsize=4)
def _get_program(n_units, n_ctx):
    return build_attn_program(n_units, n_ctx)


def _get_runner(n_units, n_ctx):
    """Build the bass program once and return a cached jitted SPMD runner:
    runner(concat_qk, concat_vv) -> concat_out  (axis 0 = cores*units)."""
    import jax
    from jax.experimental.shard_map import shard_map
    from jax.sharding import Mesh, PartitionSpec
    from concourse import bass2jax

    try:
        jax.config.update("jax_compilation_cache_dir", "/tmp/jax_neff_cache")
        jax.config.update("jax_persistent_cache_min_compile_time_secs", 10)
    except Exception:
        pass
    bass2jax.install_neuronx_cc_hook()
    nc = _get_program(n_units, n_ctx)

    in_names, out_names, out_avals, zero_shapes = [], [], [], []
    for alloc in nc.m.functions[0].allocations:
        if not isinstance(alloc, mybir.MemoryLocationSet):
            continue
        name = alloc.memorylocations[0].name
        if alloc.kind == "ExternalInput":
            if nc.partition_id_tensor is None or name != nc.partition_id_tensor.name:
                in_names.append(name)
        elif alloc.kind == "ExternalOutput":
            out_names.append(name)
            shape = tuple(alloc.tensor_shape)
            dtype = mybir.dt.np(alloc.dtype)
            out_avals.append(jax.core.ShapedArray(shape, dtype))
            zero_shapes.append((shape, dtype))
    assert in_names == ["qt", "kt", "vv"] and out_names == ["out"]
    n_params = len(in_names)
    all_names = in_names + out_names
    if nc.partition_id_tensor is not None:
        all_names.append(nc.partition_id_tensor.name)

    def _body(*args):
        operands = list(args)
        if nc.partition_id_tensor is not None:
            operands.append(bass2jax.partition_id_tensor())
        outs = bass2jax._bass_exec_p.bind(
            *operands,
            out_avals=tuple(out_avals),
            in_names=tuple(all_names),
            out_names=tuple(out_names),
            lowering_input_output_aliases=(),
            sim_require_finite=True,
            sim_require_nnan=True,
            nc=nc,
        )
        return tuple(outs)

    devices = jax.devices()[:N_CORES]
    mesh = Mesh(np.asarray(devices), ("core",))
    n_outs = len(out_names)
    sharded = jax.jit(
        shard_map(
            _body,
            mesh=mesh,
            in_specs=(PartitionSpec("core"),) * (n_params + n_outs),
            out_specs=(PartitionSpec("core"),) * n_outs,
            check_rep=False,
        ),
        keep_unused=True,
    )

    def runner(qt_all, kt_all, vv_all):
        zeros = [
            np.zeros((N_CORES * s[0], *s[1:]), dt) for (s, dt) in zero_shapes
        ]
        (out_all,) = sharded(qt_all, kt_all, vv_all, *zeros)
        return np.asarray(out_all)

    runner.sharded = sharded
    runner.mesh = mesh
    runner.zero_shapes = zero_shapes
    return runner


_RUNNERS = {}


def _run_units(unit_specs, q, k, v, n_ctx):
    """unit_specs: list of (qk_batch, head, v_batch0, v_batch1)."""
    n_units = len(unit_specs) // N_CORES
    assert n_units * N_CORES == len(unit_specs)
    key = (n_units, n_ctx)
    if key not in _RUNNERS:
        _RUNNERS[key] = _get_runner(n_units, n_ctx)
    runner = _RUNNERS[key]

    T = n_ctx // 128
    NU = N_CORES * n_units
    qt_all = np.empty((NU, 128, T * 128), np.float32)
    kt_all = np.empty((NU, 128, (T // 2) * 128), np.float32)
    import ml_dtypes
    vv_all = np.empty((NU, 128, T * 128), ml_dtypes.bfloat16)
    for i, (bq, h, b0, b1) in enumerate(unit_specs):
        qT = q[bq, h].T                      # [64, n_ctx], d-major
        qt_all[i, 0:64] = qT
        qt_all[i, 64:128] = qT               # replica feeds PE row-group 1
        kT = k[bq, h].T.reshape(64, T, 128)  # [dd, t, r]
        kt_all[i, 0:64] = kT[:, 0::2, :].reshape(64, -1)
        kt_all[i, 64:128] = kT[:, 1::2, :].reshape(64, -1)
        v0 = v[b0, h].reshape(T, 128, D)     # [t, p, dd]
        v1 = v[b1, h].reshape(T, 128, D)
        vv_all[i] = (
            np.concatenate([v0, v1], axis=2)
            .transpose(1, 0, 2)
            .reshape(128, -1)
            .astype(ml_dtypes.bfloat16)
        )

    out_all = runner(qt_all, kt_all, vv_all)  # [NU, 128, n_ctx] = out^T

    out = np.empty((B, H, n_ctx, D), np.float32)
    for i, (bq, h, b0, b1) in enumerate(unit_specs):
        out[b0, h] = out_all[i, 0:64, :].T
        if b1 != b0:
            out[b1, h] = out_all[i, 64:128, :].T
    return out


def kernel(q, k, v, swap):
    q = np.ascontiguousarray(np.asarray(q, dtype=np.float32))
    k = np.ascontiguousarray(np.asarray(k, dtype=np.float32))
    v = np.ascontiguousarray(np.asarray(v, dtype=np.float32))
    swap_val = int(np.asarray(swap).reshape(-1)[0])

    n_ctx = q.shape[2]
    if swap_val:
        # 32 pair-units: attn of (b, h) applied to v[b] and v[b + B//2]
        specs = [(bq, h, bq, bq + B // 2) for bq in range(B // 2) for h in range(H)]
    else:
        # 64 independent units (2nd v slot duplicates the 1st)
        specs = [(b, h, b, b) for b in range(B) for h in range(H)]
    return _run_units(specs, q, k, v, n_ctx)


if __name__ == "__main__":
    rng = np.random.default_rng(0)
    q = rng.standard_normal((B, H, N, D), dtype=np.float32)
    k = rng.standard_normal((B, H, N, D), dtype=np.float32)
    v = rng.standard_normal((B, H, N, D), dtype=np.float32)
    o = kernel(q, k, v, 1)
    print("out", o.shape, o.dtype, float(np.abs(o).mean()))
